# revision 1
# baseline (speedup 1.0000x reference)
"""MLA (CustomLlamaMLAForInfer) Trainium2 Bass kernel.

Sharding: tensor-parallel over heads across 8 NeuronCores. Core c owns
kv-head c and q-heads [4c, 4c+4). Every core sees the full token stream
(B*S = 4096 tokens); o_proj is computed against the core's 512
head-dims, producing a partial [4096, 4096] output that the host sums
across the 8 cores.

Device program phases (single SPMD program, per-core weights differ):
  1a. qT = Wq_shard @ hidden.T   (rope + 1/sqrt(d) folded in at evict)
  1b. c_kvT = Wdk @ hidden.T ; krT = Wkr_shard @ hidden.T (rope at evict)
  2.  k_c / v from c_kvT via Wupk/Wupv shards; assemble kT_full, v_tok
  3.  causal attention per (batch, q-head): scores_T = kT.T@qT blocks,
      exp (no max-sub needed: |scores| < ~6), mask diag blocks,
      out_T[d,q] += v_tok.T @ p_T, sums via ones-matmul, normalize
  4.  partial o_proj: out[tok, hid] += attn_T.T @ WoT_shard

All matmuls run as float32r (fp22 mantissa, 1 PE pass).
"""

import numpy as np

HIDDEN = 4096
N_HEADS = 32
KV_HEADS = 8
HEAD_DIM = 128
LOW_RANK = 64
TOP_K_ROPE = 32
ROPE_THETA = 10000.0
B, S = 2, 2048
NCORES = 8
HPC = N_HEADS // NCORES          # q heads per core = 4
QR = HPC * HEAD_DIM              # q rows per core = 512
CD = LOW_RANK * KV_HEADS         # latent dim = 512
KRR = 2 * TOP_K_ROPE             # rope rows per kv head = 64


def _rope_tables(seq_len):
    inv = 1.0 / (ROPE_THETA ** (np.arange(0, HEAD_DIM, 2, dtype=np.float32) / HEAD_DIM))
    pos = np.arange(seq_len, dtype=np.float32)
    fr = np.outer(pos, inv)
    emb = np.concatenate([fr, fr], axis=-1)          # [S, 128]
    return (np.cos(emb).T.astype(np.float32),        # [128, S]
            np.sin(emb).T.astype(np.float32))


def build_program(Bv=B, Sv=S, TB=512, QB=512, trace_sim=False):
    """Build the SPMD Bass program. TB = proj token-block, QB = attention
    q-block (both <= 512, the fp32 moving-operand limit)."""
    from concourse import bacc, tile, mybir
    import concourse.bass as bass

    f32 = mybir.dt.float32
    F32R = mybir.dt.float32r
    MS = bass.MemorySpace
    EXP = mybir.ActivationFunctionType.Exp

    NT = Bv * Sv                 # total tokens
    HT = HIDDEN // 128           # hidden tiles = 32
    NTB = NT // TB               # proj token blocks
    NQB = Sv // QB               # q blocks per batch
    NJ = QB // 128               # diagonal mask variants
    NKT_B = Sv // 128            # k tiles per batch
    QT = QR // 128               # q-head tiles per core = 4
    LT = CD // 128               # latent tiles = 4

    nc = bacc.Bacc("TRN2", target_bir_lowering=False, debug=False,
                   num_devices=NCORES)

    def din(name, shape):
        return nc.dram_tensor(name, shape, f32, kind="ExternalInput").ap()

    hidT = din("hidT", [HIDDEN, NT])
    wq = din("wq_t", [HIDDEN, QR])
    wkd = din("wkd_t", [HIDDEN, CD + KRR])
    wupk = din("wupk_t", [CD, KRR])
    wupv = din("wupv_t", [CD, HEAD_DIM])
    wo = din("wo_t", [QR, HIDDEN])
    qcos = din("qcos", [128, NT])
    qsin = din("qsin", [128, NT])
    kcos = din("kcos", [KRR, NT])
    ksin = din("ksin", [KRR, NT])
    masks = din("masks", [128, NJ, QB])
    onesd = din("ones", [128, 1])
    outp = nc.dram_tensor("out_part", [NT, HIDDEN], f32, kind="ExternalOutput").ap()
    qT_s = nc.dram_tensor("qT_s", [QT, 128, NT], f32).ap()
    ckv_s = nc.dram_tensor("ckv_s", [LT, 128, NT], f32).ap()

    with tile.TileContext(nc, trace_sim=trace_sim) as tc:
        with tc.tile_pool(name="persist", bufs=1) as pers:
            kT_full = pers.tile([128, NT], F32R, tag="kT")
            v_tok = pers.tile([128, NT // 128, HEAD_DIM], F32R, tag="vtok")

            # ---------------- phase 1: projections of hidden ----------------
            with tc.tile_pool(name="cos", bufs=1) as cp:
                qcos_sb = cp.tile([128, NT], f32, tag="qc")
                qsin_sb = cp.tile([128, NT], f32, tag="qs")
                kcos_sb = cp.tile([KRR, NT], f32, tag="kc")
                ksin_sb = cp.tile([KRR, NT], f32, tag="ks")
                nc.sync.dma_start(qcos_sb[:], qcos)
                nc.sync.dma_start(qsin_sb[:], qsin)
                nc.sync.dma_start(kcos_sb[:], kcos)
                nc.sync.dma_start(ksin_sb[:], ksin)

                # ---- pass A: q projection (+rope, +1/sqrt(d) via tables) ----
                with tc.tile_pool(name="wqp", bufs=1) as wqp, \
                     tc.tile_pool(name="hidA", bufs=8) as hpA, \
                     tc.tile_pool(name="stA", bufs=2) as stA, \
                     tc.tile_pool(name="psA", bufs=8, space=MS.PSUM) as ppA:
                    wq_sb = wqp.tile([128, HT, QR], F32R)
                    nc.sync.dma_start(wq_sb[:], wq.rearrange("(t p) w -> p t w", p=128).bitcast(F32R))
                    for blk in range(NTB):
                        c0, c1 = blk * TB, (blk + 1) * TB
                        qps = [ppA.tile([128, TB], f32, tag="qps", name=f"qps{_m}") for _m in range(QT)]
                        for t in range(HT):
                            ht = hpA.tile([128, TB], F32R, tag="hid")
                            nc.sync.dma_start(ht[:], hidT[t * 128:(t + 1) * 128, c0:c1].bitcast(F32R))
                            for m in range(QT):
                                nc.tensor.matmul(
                                    qps[m][:],
                                    wq_sb[:, t, m * 128:(m + 1) * 128],
                                    ht[:],
                                    start=(t == 0), stop=(t == HT - 1))
                        for m in range(QT):
                            raw = stA.tile([128, TB], f32, tag="raw")
                            nc.scalar.copy(raw[:], qps[m][:])
                            rot = stA.tile([128, TB], f32, tag="rot")
                            nc.sync.dma_start(rot[0:64, :], raw[64:128, :])
                            nc.sync.dma_start(rot[64:128, :], raw[0:64, :])
                            qsb = stA.tile([128, TB], f32, tag="qsb")
                            nc.vector.tensor_mul(qsb[:], raw[:], qcos_sb[:, c0:c1])
                            nc.vector.tensor_mul(rot[:], rot[:], qsin_sb[:, c0:c1])
                            nc.vector.tensor_add(qsb[:], qsb[:], rot[:])
                            nc.sync.dma_start(qT_s[m, :, c0:c1], qsb[:])

                # ---- pass B: c_kv (latent) + k_rope projections ----
                with tc.tile_pool(name="wkdp", bufs=1) as wkdp, \
                     tc.tile_pool(name="hidB", bufs=8) as hpB, \
                     tc.tile_pool(name="stB", bufs=2) as stB, \
                     tc.tile_pool(name="psB", bufs=6, space=MS.PSUM) as ppB, \
                     tc.tile_pool(name="psBk", bufs=2, space=MS.PSUM) as ppBk:
                    wkd_sb = wkdp.tile([128, HT, CD + KRR], F32R)
                    nc.sync.dma_start(wkd_sb[:], wkd.rearrange("(t p) w -> p t w", p=128).bitcast(F32R))
                    for blk in range(NTB):
                        c0, c1 = blk * TB, (blk + 1) * TB
                        dps = [ppB.tile([128, TB], f32, tag="dps", name=f"dps{_m}") for _m in range(LT)]
                        krp = ppBk.tile([KRR, TB], f32, tag="krp")
                        for t in range(HT):
                            ht = hpB.tile([128, TB], F32R, tag="hid")
                            nc.sync.dma_start(ht[:], hidT[t * 128:(t + 1) * 128, c0:c1].bitcast(F32R))
                            for m in range(LT):
                                nc.tensor.matmul(
                                    dps[m][:],
                                    wkd_sb[:, t, m * 128:(m + 1) * 128],
                                    ht[:],
                                    start=(t == 0), stop=(t == HT - 1))
                            nc.tensor.matmul(
                                krp[:],
                                wkd_sb[:, t, CD:CD + KRR],
                                ht[:],
                                start=(t == 0), stop=(t == HT - 1))
                        for m in range(LT):
                            csb = stB.tile([128, TB], f32, tag="csb")
                            nc.scalar.copy(csb[:], dps[m][:])
                            nc.sync.dma_start(ckv_s[m, :, c0:c1], csb[:])
                        # rope the 64 k-rope rows, scatter into kT_full
                        rawk = stB.tile([KRR, TB], f32, tag="rawk")
                        nc.scalar.copy(rawk[:], krp[:])
                        rotk = stB.tile([KRR, TB], f32, tag="rotk")
                        nc.sync.dma_start(rotk[0:32, :], rawk[32:64, :])
                        nc.sync.dma_start(rotk[32:64, :], rawk[0:32, :])
                        ksb = stB.tile([KRR, TB], f32, tag="ksb")
                        nc.vector.tensor_mul(ksb[:], rawk[:], kcos_sb[:, c0:c1])
                        nc.vector.tensor_mul(rotk[:], rotk[:], ksin_sb[:, c0:c1])
                        nc.vector.tensor_add(ksb[:], ksb[:], rotk[:])
                        nc.sync.dma_start(kT_full[0:32, c0:c1], ksb[0:32, :].bitcast(F32R))
                        nc.sync.dma_start(kT_full[64:96, c0:c1], ksb[32:64, :].bitcast(F32R))

            # ---------------- phase 2: k_c and v from the latent ----------------
            with tc.tile_pool(name="wup", bufs=1) as wup, \
                 tc.tile_pool(name="ckvb", bufs=2) as ckvb, \
                 tc.tile_pool(name="st2", bufs=2) as st2, \
                 tc.tile_pool(name="psK", bufs=2, space=MS.PSUM) as psK, \
                 tc.tile_pool(name="psV", bufs=4, space=MS.PSUM) as psV:
                wupk_sb = wup.tile([128, LT, KRR], F32R, tag="upk")
                wupv_sb = wup.tile([128, LT, HEAD_DIM], F32R, tag="upv")
                nc.sync.dma_start(wupk_sb[:], wupk.rearrange("(t p) w -> p t w", p=128).bitcast(F32R))
                nc.sync.dma_start(wupv_sb[:], wupv.rearrange("(t p) w -> p t w", p=128).bitcast(F32R))
                for blk in range(NTB):
                    c0, c1 = blk * TB, (blk + 1) * TB
                    cb = ckvb.tile([128, LT, TB], F32R, tag="cb")
                    nc.sync.dma_start(cb[:], ckv_s[:, :, c0:c1].rearrange("t p w -> p t w").bitcast(F32R))
                    kcp = psK.tile([KRR, TB], f32, tag="kcp")
                    for lt in range(LT):
                        nc.tensor.matmul(kcp[:],
                                         wupk_sb[:, lt, :],
                                         cb[:, lt, :],
                                         start=(lt == 0), stop=(lt == LT - 1))
                    kcs = st2.tile([KRR, TB], f32, tag="kcs")
                    nc.scalar.copy(kcs[:], kcp[:])
                    nc.sync.dma_start(kT_full[32:64, c0:c1], kcs[0:32, :].bitcast(F32R))
                    nc.sync.dma_start(kT_full[96:128, c0:c1], kcs[32:64, :].bitcast(F32R))
                    for tt in range(TB // 128):
                        vp = psV.tile([128, HEAD_DIM], f32, tag="vp")
                        for lt in range(LT):
                            nc.tensor.matmul(
                                vp[:],
                                cb[:, lt, tt * 128:(tt + 1) * 128],
                                wupv_sb[:, lt, :],
                                start=(lt == 0), stop=(lt == LT - 1))
                        nc.scalar.copy(v_tok[:, blk * (TB // 128) + tt, :], vp[:])

            # ---------------- phases 3+4 ----------------
            with tc.tile_pool(name="attn", bufs=1) as ap_:
                attn_sb = ap_.tile([128, QT, NT], F32R)

                with tc.tile_pool(name="qh", bufs=2) as qhp, \
                     tc.tile_pool(name="cst3", bufs=1) as cst3, \
                     tc.tile_pool(name="pt", bufs=3) as ptp, \
                     tc.tile_pool(name="sm", bufs=2) as smp, \
                     tc.tile_pool(name="psS", bufs=3, space=MS.PSUM) as psS, \
                     tc.tile_pool(name="psO", bufs=2, space=MS.PSUM) as psO, \
                     tc.tile_pool(name="psU", bufs=2, space=MS.PSUM) as psU:
                    masks_sb = cst3.tile([128, NJ, QB], F32R, tag="masks")
                    nc.sync.dma_start(masks_sb[:], masks.bitcast(F32R))
                    ones_sb = cst3.tile([128, 1], F32R, tag="ones")
                    nc.sync.dma_start(ones_sb[:], onesd.bitcast(F32R))
                    for h in range(QT):
                        qh_sb = qhp.tile([128, NT], F32R, tag="qh")
                        nc.sync.dma_start(qh_sb[:], qT_s[h].bitcast(F32R))
                        for b in range(Bv):
                            off = b * Sv
                            for qb in range(NQB):
                                ops = psO.tile([128, QB], f32, tag="ops")
                                sps = psU.tile([1, QB], f32, tag="sps")
                                nkt = (qb + 1) * NJ
                                for kt in range(nkt):
                                    scp = psS.tile([128, QB], f32, tag="scp")
                                    nc.tensor.matmul(
                                        scp[:],
                                        kT_full[:, off + kt * 128: off + (kt + 1) * 128],
                                        qh_sb[:, off + qb * QB: off + (qb + 1) * QB],
                                        start=True, stop=True)
                                    ptile = ptp.tile([128, QB], F32R, tag="pt")
                                    nc.scalar.activation(ptile[:], scp[:], EXP)
                                    j = kt - qb * NJ
                                    if j >= 0:
                                        nc.vector.tensor_mul(ptile[:], ptile[:], masks_sb[:, j, :])
                                    nc.tensor.matmul(
                                        ops[:],
                                        v_tok[:, b * NKT_B + kt, :],
                                        ptile[:],
                                        start=(kt == 0), stop=(kt == nkt - 1))
                                    nc.tensor.matmul(
                                        sps[:],
                                        ones_sb[:],
                                        ptile[:],
                                        start=(kt == 0), stop=(kt == nkt - 1))
                                rec = smp.tile([1, QB], f32, tag="rec")
                                nc.vector.reciprocal(rec[:], sps[:])
                                rb = smp.tile([128, QB], f32, tag="rb")
                                nc.gpsimd.partition_broadcast(rb[:], rec[:])
                                nc.vector.tensor_mul(
                                    attn_sb[:, h, off + qb * QB: off + (qb + 1) * QB],
                                    ops[:], rb[:])

                # ---- phase 4: partial o_proj ----
                with tc.tile_pool(name="wop", bufs=1) as wop, \
                     tc.tile_pool(name="st4", bufs=4) as st4, \
                     tc.tile_pool(name="ps4", bufs=6, space=MS.PSUM) as ps4:
                    wo_sb = wop.tile([128, QT, HIDDEN], F32R)
                    nc.sync.dma_start(wo_sb[:], wo.rearrange("(t p) w -> p t w", p=128).bitcast(F32R))
                    for T in range(NT // 128):
                        for n in range(HIDDEN // 512):
                            ps = ps4.tile([128, 512], f32, tag="ps")
                            for h2 in range(QT):
                                nc.tensor.matmul(
                                    ps[:],
                                    attn_sb[:, h2, T * 128:(T + 1) * 128],
                                    wo_sb[:, h2, n * 512:(n + 1) * 512],
                                    start=(h2 == 0), stop=(h2 == QT - 1))
                            osb = st4.tile([128, 512], f32, tag="osb")
                            nc.vector.tensor_copy(osb[:], ps[:])
                            nc.sync.dma_start(outp[T * 128:(T + 1) * 128, n * 512:(n + 1) * 512], osb[:])

    nc.compile()
    return nc


def make_in_maps(hidden_states, Wq, Wkr, Wdk, Wupk, Wupv, Wo, Bv=B, Sv=S, QB=512):
    """Host-side sharding + layout prep. Returns per-core input dicts."""
    NT = Bv * Sv
    NJ = QB // 128
    scale = 1.0 / np.sqrt(np.float32(HEAD_DIM))

    hidT = np.ascontiguousarray(
        hidden_states.reshape(NT, HIDDEN).T.astype(np.float32))

    cos_t, sin_t = _rope_tables(Sv)                    # [128, S]
    cos_t = np.tile(cos_t, (1, Bv))                    # [128, NT]
    sin_t = np.tile(sin_t, (1, Bv))
    qcos = np.ascontiguousarray(cos_t * scale)
    qsin = np.ascontiguousarray(
        np.concatenate([-sin_t[0:64], sin_t[64:128]], axis=0) * scale)
    kcos = np.ascontiguousarray(
        np.concatenate([cos_t[0:32], cos_t[64:96]], axis=0))
    ksin = np.ascontiguousarray(
        np.concatenate([-sin_t[0:32], sin_t[64:96]], axis=0))

    k_idx = np.arange(128)[:, None]
    q_idx = np.arange(QB)[None, :]
    masks = np.stack(
        [(q_idx >= j * 128 + k_idx).astype(np.float32) for j in range(NJ)],
        axis=1)                                        # [128, NJ, QB]
    masks = np.ascontiguousarray(masks)

    in_maps = []
    for c in range(NCORES):
        wq_t = np.ascontiguousarray(Wq[QR * c:QR * (c + 1)].T.astype(np.float32))
        wkd_t = np.ascontiguousarray(
            np.concatenate([Wdk, Wkr[KRR * c:KRR * (c + 1)]], axis=0).T.astype(np.float32))
        wupk_t = np.ascontiguousarray(Wupk[KRR * c:KRR * (c + 1)].T.astype(np.float32))
        wupv_t = np.ascontiguousarray(
            Wupv[HEAD_DIM * c:HEAD_DIM * (c + 1)].T.astype(np.float32))
        wo_t = np.ascontiguousarray(Wo[:, QR * c:QR * (c + 1)].T.astype(np.float32))
        in_maps.append({
            "hidT": hidT, "wq_t": wq_t, "wkd_t": wkd_t,
            "wupk_t": wupk_t, "wupv_t": wupv_t, "wo_t": wo_t,
            "qcos": qcos, "qsin": qsin, "kcos": kcos, "ksin": ksin,
            "masks": masks, "ones": np.ones((128, 1), np.float32),
        })
    return in_maps


_NC_CACHE = {}


def _get_program(key=(B, S, 512, 512)):
    if key not in _NC_CACHE:
        _NC_CACHE[key] = build_program(*key)
    return _NC_CACHE[key]


def kernel(hidden_states, Wq, Wkr, Wdk, Wupk, Wupv, Wo):
    from concourse.bass_utils import run_bass_kernel_spmd

    hidden_states = np.asarray(hidden_states)
    in_maps = make_in_maps(hidden_states, np.asarray(Wq), np.asarray(Wkr),
                           np.asarray(Wdk), np.asarray(Wupk), np.asarray(Wupv),
                           np.asarray(Wo))
    nc = _get_program()
    res = run_bass_kernel_spmd(nc, in_maps, list(range(NCORES)))
    out = res.results[0]["out_part"].astype(np.float32)
    for i in range(1, NCORES):
        out = out + res.results[i]["out_part"]
    return out.reshape(B, S, HIDDEN).astype(np.float32)



# revision 5
# speedup vs baseline: 1.1829x; 1.1829x over previous
"""MLA (CustomLlamaMLAForInfer) Trainium2 Bass kernel, v2.

Sharding: batch x tensor-parallel. Core c owns batch c//4 and TP shard
c%4: 8 q-heads [8*(c%4), 8*(c%4)+8), 2 kv-heads [2*(c%4), 2*(c%4)+2),
and the matching 1024 columns of Wo. Each core sees its batch's 2048
tokens. o_proj partials ([HIDDEN, 2048] fp16, transposed layout) are
summed per batch group of 4 on the host.

All matmul operands are bf16 (fp32 PSUM accumulation); rope math runs
in fp32 on the DVE at PSUM-evict time. Softmax denominators are
accumulated on the DVE (fp32) and reduced across partitions with a
single ones-matmul per (head, q-block).

Device phases (single SPMD program; per-core weights differ):
  1. per 512-token block: c_kv/k_rope projections (5 PSUM tiles),
     q projection (8 PSUM tiles), rope at evict, then k_nope/v from
     the block latent. k/v stay in SBUF; qT spills to DRAM (bf16).
  2. causal attention per (q-head, q-block): scores_T = kT.T @ qT,
     exp (|scores| bounded, no max-sub), diag-block masks, PV via
     v-stationary matmuls, DVE-accumulated denominators.
  3. partial o_proj in [hid, tok] layout, fp16 output.
"""

import numpy as np
import ml_dtypes

HIDDEN = 4096
N_HEADS = 32
KV_HEADS = 8
HEAD_DIM = 128
LOW_RANK = 64
TOP_K_ROPE = 32
ROPE_THETA = 10000.0
B, S = 2, 2048
NCORES = 8
TPG = 4                      # TP group size (cores per batch)
HPC = N_HEADS // TPG         # q heads per core = 8
KVPC = KV_HEADS // TPG       # kv heads per core = 2
QR = HPC * HEAD_DIM          # q rows per core = 1024
CD = LOW_RANK * KV_HEADS     # latent dim = 512
KRR = 64 * KVPC              # rope rows per core = 128

BF16 = ml_dtypes.bfloat16


def _rope_tables(seq_len):
    inv = 1.0 / (ROPE_THETA ** (np.arange(0, HEAD_DIM, 2, dtype=np.float32) / HEAD_DIM))
    pos = np.arange(seq_len, dtype=np.float32)
    fr = np.outer(pos, inv)
    emb = np.concatenate([fr, fr], axis=-1)          # [S, 128]
    return (np.cos(emb).T.astype(np.float32),        # [128, S]
            np.sin(emb).T.astype(np.float32))


def build_program(Sv=S, TB=512, QB=512):
    from concourse import bacc, tile, mybir
    import concourse.bass as bass

    f32 = mybir.dt.float32
    BF = mybir.dt.bfloat16
    F16 = mybir.dt.float16
    F32R = mybir.dt.float32r
    MS = bass.MemorySpace
    EXP = mybir.ActivationFunctionType.Exp

    NT = Sv                      # tokens per core (one batch)
    HT = HIDDEN // 128           # hidden tiles = 32
    NTB = NT // TB               # proj token blocks = 4
    NQB = NT // QB               # attention q blocks = 4
    NJ = QB // 128               # diagonal mask variants = 4
    QT = HPC                     # q-head tiles = 8
    LT = CD // 128               # latent tiles = 4
    NKT = NT // 128              # k tiles = 16

    nc = bacc.Bacc("TRN2", target_bir_lowering=False, debug=False,
                   num_devices=NCORES)

    def din(name, shape, dt=BF):
        return nc.dram_tensor(name, shape, dt, kind="ExternalInput").ap()

    hidT = din("hidT", [HIDDEN, NT])
    wq = din("wq_t", [HIDDEN, QR])
    wkd = din("wkd_t", [HIDDEN, CD + KRR])
    wupk = din("wupk_t", [CD, KRR])
    wupv = din("wupv_t", [CD, KVPC * HEAD_DIM])
    wo = din("wo_t", [QR, HIDDEN])
    qcos = din("qcos", [128, NT])
    qsin = din("qsin", [128, NT])
    kcos = din("kcos", [128, NT])
    ksin = din("ksin", [128, NT])
    masks = din("masks", [128, NJ, QB])
    onesd = din("ones", [128, 1], f32)
    outp = nc.dram_tensor("out_part", [HIDDEN, NT], F16, kind="ExternalOutput").ap()
    qT_s = nc.dram_tensor("qT_s", [QT, 128, NT], BF).ap()

    with tile.TileContext(nc) as tc:
        with tc.tile_pool(name="persist", bufs=1) as pers:
            kT = pers.tile([128, KVPC, NT], BF, tag="kT")
            v_sb = pers.tile([128, NKT, KVPC, HEAD_DIM], BF, tag="v")

            # ---------------- phase 1: projections ----------------
            with tc.tile_pool(name="tabs", bufs=1) as tbp, \
                 tc.tile_pool(name="w1", bufs=1) as w1, \
                 tc.tile_pool(name="hb", bufs=1) as hbp, \
                 tc.tile_pool(name="cbp", bufs=2) as cbp, \
                 tc.tile_pool(name="st1", bufs=2) as st1, \
                 tc.tile_pool(name="ps1", bufs=8, space=MS.PSUM) as ps1:
                qcos_sb = tbp.tile([128, NT], BF, tag="qc")
                qsin_sb = tbp.tile([128, NT], BF, tag="qs")
                kcos_sb = tbp.tile([128, NT], BF, tag="kc")
                ksin_sb = tbp.tile([128, NT], BF, tag="ks")
                nc.sync.dma_start(qcos_sb[:], qcos)
                nc.sync.dma_start(qsin_sb[:], qsin)
                nc.sync.dma_start(kcos_sb[:], kcos)
                nc.sync.dma_start(ksin_sb[:], ksin)

                wq_sb = w1.tile([128, HT, QR], BF, tag="wq")
                wkd_sb = w1.tile([128, HT, CD + KRR], BF, tag="wkd")
                wupk_sb = w1.tile([128, LT, KRR], BF, tag="upk")
                wupv_sb = w1.tile([128, LT, KVPC * HEAD_DIM], BF, tag="upv")
                nc.sync.dma_start(wq_sb[:], wq.rearrange("(t p) w -> p t w", p=128))
                nc.sync.dma_start(wkd_sb[:], wkd.rearrange("(t p) w -> p t w", p=128))
                nc.sync.dma_start(wupk_sb[:], wupk.rearrange("(t p) w -> p t w", p=128))
                nc.sync.dma_start(wupv_sb[:], wupv.rearrange("(t p) w -> p t w", p=128))

                for blk in range(NTB):
                    c0, c1 = blk * TB, (blk + 1) * TB
                    hblk = hbp.tile([128, HT, TB], BF, tag="hid")
                    nc.sync.dma_start(
                        hblk[:], hidT[:, c0:c1].rearrange("(t p) w -> p t w", p=128))

                    # kd sub-pass: latent (4 tiles) + k-rope (1 tile)
                    dps = [ps1.tile([128, TB], f32, tag="ps", name=f"dps{_m}")
                           for _m in range(LT)]
                    krp = ps1.tile([128, TB], f32, tag="ps", name="krp")
                    for t in range(HT):
                        for m in range(LT):
                            nc.tensor.matmul(
                                dps[m][:], wkd_sb[:, t, m * 128:(m + 1) * 128],
                                hblk[:, t, :], start=(t == 0), stop=(t == HT - 1))
                        nc.tensor.matmul(
                            krp[:], wkd_sb[:, t, CD:CD + KRR],
                            hblk[:, t, :], start=(t == 0), stop=(t == HT - 1))

                    cb = cbp.tile([128, LT, TB], BF, tag="cb")
                    for m in range(LT):
                        nc.scalar.copy(cb[:, m, :], dps[m][:])
                    # k-rope rows: per kv head 64 rows = [dims 0:32, dims 64:96]
                    rawk = st1.tile([128, TB], f32, tag="rawk")
                    nc.scalar.copy(rawk[:], krp[:])
                    rotk = st1.tile([128, TB], f32, tag="rotk")
                    nc.sync.dma_start(rotk[0:32, :], rawk[32:64, :])
                    nc.sync.dma_start(rotk[32:64, :], rawk[0:32, :])
                    nc.sync.dma_start(rotk[64:96, :], rawk[96:128, :])
                    nc.sync.dma_start(rotk[96:128, :], rawk[64:96, :])
                    nc.vector.tensor_mul(rawk[:], rawk[:], kcos_sb[:, c0:c1])
                    nc.vector.tensor_mul(rotk[:], rotk[:], ksin_sb[:, c0:c1])
                    for kv in range(KVPC):
                        r0 = kv * 64
                        nc.vector.tensor_add(kT[0:32, kv, c0:c1],
                                             rawk[r0:r0 + 32, :], rotk[r0:r0 + 32, :])
                        nc.vector.tensor_add(kT[64:96, kv, c0:c1],
                                             rawk[r0 + 32:r0 + 64, :], rotk[r0 + 32:r0 + 64, :])

                    # q sub-pass: 8 head tiles
                    qps = [ps1.tile([128, TB], f32, tag="ps", name=f"qps{_m}")
                           for _m in range(QT)]
                    for t in range(HT):
                        for m in range(QT):
                            nc.tensor.matmul(
                                qps[m][:], wq_sb[:, t, m * 128:(m + 1) * 128],
                                hblk[:, t, :], start=(t == 0), stop=(t == HT - 1))
                    for m in range(QT):
                        raw = st1.tile([128, TB], f32, tag="qraw")
                        nc.scalar.copy(raw[:], qps[m][:])
                        rot = st1.tile([128, TB], f32, tag="qrot")
                        nc.sync.dma_start(rot[0:64, :], raw[64:128, :])
                        nc.sync.dma_start(rot[64:128, :], raw[0:64, :])
                        nc.vector.tensor_mul(raw[:], raw[:], qcos_sb[:, c0:c1])
                        nc.vector.tensor_mul(rot[:], rot[:], qsin_sb[:, c0:c1])
                        qsb = st1.tile([128, TB], BF, tag="qsb")
                        nc.vector.tensor_add(qsb[:], raw[:], rot[:])
                        nc.sync.dma_start(qT_s[m, :, c0:c1], qsb[:])

                    # up-projections from the block latent
                    kcp = ps1.tile([128, TB], f32, tag="ps", name="kcp")
                    for lt in range(LT):
                        nc.tensor.matmul(kcp[:], wupk_sb[:, lt, :], cb[:, lt, :],
                                         start=(lt == 0), stop=(lt == LT - 1))
                    # rows: [kv0 d32:64, kv0 d96:128, kv1 d32:64, kv1 d96:128]
                    for kv in range(KVPC):
                        r0 = kv * 64
                        nc.scalar.copy(kT[32:64, kv, c0:c1], kcp[r0:r0 + 32, :])
                        nc.scalar.copy(kT[96:128, kv, c0:c1], kcp[r0 + 32:r0 + 64, :])
                    for tt in range(TB // 128):
                        vp = ps1.tile([128, TB], f32, tag="ps", name=f"vp{tt}")
                        for lt in range(LT):
                            nc.tensor.matmul(
                                vp[:, 0:KVPC * HEAD_DIM],
                                cb[:, lt, tt * 128:(tt + 1) * 128],
                                wupv_sb[:, lt, :],
                                start=(lt == 0), stop=(lt == LT - 1))
                        nc.scalar.copy(v_sb[:, blk * (TB // 128) + tt, :, :],
                                       vp[:, 0:KVPC * HEAD_DIM])

            # ---------------- phase 2: attention ----------------
            with tc.tile_pool(name="wop", bufs=1) as wop, \
                 tc.tile_pool(name="att", bufs=1) as ap_:
                wo_sb = wop.tile([128, QT, HIDDEN], BF, tag="wo")
                nc.sync.dma_start(wo_sb[:], wo.rearrange("(t p) w -> p t w", p=128))
                attn_sb = ap_.tile([128, QT, NT], BF, tag="attn")

                with tc.tile_pool(name="cst", bufs=1) as cst, \
                     tc.tile_pool(name="qh", bufs=2) as qhp, \
                     tc.tile_pool(name="pt", bufs=4) as ptp, \
                     tc.tile_pool(name="sac", bufs=2) as sap, \
                     tc.tile_pool(name="sm", bufs=2) as smp, \
                     tc.tile_pool(name="psS", bufs=3, space=MS.PSUM) as psS, \
                     tc.tile_pool(name="psO", bufs=2, space=MS.PSUM) as psO, \
                     tc.tile_pool(name="psU", bufs=2, space=MS.PSUM) as psU:
                    masks_sb = cst.tile([128, NJ, QB], BF, tag="masks")
                    nc.sync.dma_start(masks_sb[:], masks)
                    ones_sb = cst.tile([128, 1], F32R, tag="ones")
                    nc.sync.dma_start(ones_sb[:], onesd.bitcast(F32R))
                    for m in range(QT):
                        kvh = m // (HPC // KVPC)
                        qh = qhp.tile([128, NT], BF, tag="qh")
                        nc.sync.dma_start(qh[:], qT_s[m])
                        for qb in range(NQB):
                            nkt = (qb + 1) * NJ
                            ops = psO.tile([128, QB], f32, tag="ops")
                            sacc = sap.tile([128, QB], F32R, tag="sacc")
                            for kt in range(nkt):
                                scp = psS.tile([128, QB], f32, tag="scp")
                                nc.tensor.matmul(
                                    scp[:],
                                    kT[:, kvh, kt * 128:(kt + 1) * 128],
                                    qh[:, qb * QB:(qb + 1) * QB],
                                    start=True, stop=True)
                                ptile = ptp.tile([128, QB], BF, tag="pt")
                                nc.scalar.activation(ptile[:], scp[:], EXP)
                                j = kt - qb * NJ
                                if j >= 0:
                                    nc.vector.tensor_mul(ptile[:], ptile[:],
                                                         masks_sb[:, j, :])
                                nc.tensor.matmul(
                                    ops[:], v_sb[:, kt, kvh, :], ptile[:],
                                    start=(kt == 0), stop=(kt == nkt - 1))
                                if kt == 0:
                                    nc.vector.tensor_copy(sacc[:], ptile[:])
                                else:
                                    nc.vector.tensor_add(sacc[:], sacc[:], ptile[:])
                            sps = psU.tile([1, QB], f32, tag="sps")
                            nc.tensor.matmul(sps[:], ones_sb[:], sacc[:],
                                             start=True, stop=True)
                            rec = smp.tile([1, QB], f32, tag="rec")
                            nc.vector.reciprocal(rec[:], sps[:])
                            rb = smp.tile([128, QB], f32, tag="rb")
                            nc.gpsimd.partition_broadcast(rb[:], rec[:])
                            nc.vector.tensor_mul(
                                attn_sb[:, m, qb * QB:(qb + 1) * QB],
                                ops[:], rb[:])

                # ---------------- phase 3: partial o_proj ----------------
                with tc.tile_pool(name="st4", bufs=4) as st4, \
                     tc.tile_pool(name="ps4", bufs=8, space=MS.PSUM) as ps4:
                    for n in range(HT):
                        for j in range(NQB):
                            ps = ps4.tile([128, QB], f32, tag="ps")
                            for h2 in range(QT):
                                nc.tensor.matmul(
                                    ps[:],
                                    wo_sb[:, h2, n * 128:(n + 1) * 128],
                                    attn_sb[:, h2, j * QB:(j + 1) * QB],
                                    start=(h2 == 0), stop=(h2 == QT - 1))
                            osb = st4.tile([128, QB], F16, tag="osb")
                            nc.vector.tensor_copy(osb[:], ps[:])
                            nc.sync.dma_start(
                                outp[n * 128:(n + 1) * 128, j * QB:(j + 1) * QB],
                                osb[:])

    nc.compile()
    return nc


def make_in_maps(hidden_states, Wq, Wkr, Wdk, Wupk, Wupv, Wo, Sv=S, QB=512):
    """Host-side sharding + layout prep. Returns per-core input dicts."""
    NJ = QB // 128
    scale = 1.0 / np.sqrt(np.float32(HEAD_DIM))
    hidden_states = np.asarray(hidden_states, np.float32)
    Wq, Wkr, Wdk = np.asarray(Wq, np.float32), np.asarray(Wkr, np.float32), np.asarray(Wdk, np.float32)
    Wupk, Wupv, Wo = np.asarray(Wupk, np.float32), np.asarray(Wupv, np.float32), np.asarray(Wo, np.float32)

    cos_t, sin_t = _rope_tables(Sv)                    # [128, S]
    qcos = np.ascontiguousarray(cos_t * scale).astype(BF16)
    qsin = np.ascontiguousarray(
        np.concatenate([-sin_t[0:64], sin_t[64:128]], axis=0) * scale).astype(BF16)
    # per kv head 64 rope rows = [dims 0:32, dims 64:96], tiled x KVPC
    kc1 = np.concatenate([cos_t[0:32], cos_t[64:96]], axis=0)
    ks1 = np.concatenate([-sin_t[0:32], sin_t[64:96]], axis=0)
    kcos = np.ascontiguousarray(np.tile(kc1, (KVPC, 1))).astype(BF16)
    ksin = np.ascontiguousarray(np.tile(ks1, (KVPC, 1))).astype(BF16)

    k_idx = np.arange(128)[:, None]
    q_idx = np.arange(QB)[None, :]
    masks = np.stack(
        [(q_idx >= j * 128 + k_idx).astype(np.float32) for j in range(NJ)],
        axis=1)                                        # [128, NJ, QB]
    masks = np.ascontiguousarray(masks).astype(BF16)

    in_maps = []
    for c in range(NCORES):
        b, tp = c // TPG, c % TPG
        hidT = np.ascontiguousarray(hidden_states[b].T).astype(BF16)
        wq_t = np.ascontiguousarray(Wq[QR * tp:QR * (tp + 1)].T).astype(BF16)
        wkd_t = np.ascontiguousarray(
            np.concatenate([Wdk, Wkr[KRR * tp:KRR * (tp + 1)]], axis=0).T).astype(BF16)
        wupk_t = np.ascontiguousarray(Wupk[KRR * tp:KRR * (tp + 1)].T).astype(BF16)
        wupv_t = np.ascontiguousarray(
            Wupv[KVPC * HEAD_DIM * tp:KVPC * HEAD_DIM * (tp + 1)].T).astype(BF16)
        wo_t = np.ascontiguousarray(Wo[:, QR * tp:QR * (tp + 1)].T).astype(BF16)
        in_maps.append({
            "hidT": hidT, "wq_t": wq_t, "wkd_t": wkd_t,
            "wupk_t": wupk_t, "wupv_t": wupv_t, "wo_t": wo_t,
            "qcos": qcos, "qsin": qsin, "kcos": kcos, "ksin": ksin,
            "masks": masks, "ones": np.ones((128, 1), np.float32),
        })
    return in_maps


def combine_outputs(res):
    outs = []
    for b in range(B):
        acc = res.results[b * TPG]["out_part"].astype(np.float32)
        for tp in range(1, TPG):
            acc = acc + res.results[b * TPG + tp]["out_part"].astype(np.float32)
        outs.append(acc.T)                             # [S, HIDDEN]
    return np.stack(outs).astype(np.float32)           # [B, S, HIDDEN]


_NC_CACHE = {}


def _get_program(key=(S, 512, 512)):
    if key not in _NC_CACHE:
        _NC_CACHE[key] = build_program(*key)
    return _NC_CACHE[key]


def kernel(hidden_states, Wq, Wkr, Wdk, Wupk, Wupv, Wo):
    from concourse.bass_utils import run_bass_kernel_spmd

    in_maps = make_in_maps(np.asarray(hidden_states), Wq, Wkr, Wdk, Wupk, Wupv, Wo)
    nc = _get_program()
    res = run_bass_kernel_spmd(nc, in_maps, list(range(NCORES)))
    return combine_outputs(res)


# revision 10
# speedup vs baseline: 1.2634x; 1.0681x over previous
"""MLA (CustomLlamaMLAForInfer) Trainium2 Bass kernel, v2.

Sharding: batch x tensor-parallel. Core c owns batch c//4 and TP shard
c%4: 8 q-heads [8*(c%4), 8*(c%4)+8), 2 kv-heads [2*(c%4), 2*(c%4)+2),
and the matching 1024 columns of Wo. Each core sees its batch's 2048
tokens. o_proj partials ([HIDDEN, 2048] fp16, transposed layout) are
summed per batch group of 4 on the host.

All matmul operands are bf16 (fp32 PSUM accumulation); rope math runs
in fp32 on the DVE at PSUM-evict time. Softmax denominators are
accumulated on the DVE (fp32) and reduced across partitions with a
single ones-matmul per (head, q-block).

Device phases (single SPMD program; per-core weights differ):
  1. per 512-token block: c_kv/k_rope projections (5 PSUM tiles),
     q projection (8 PSUM tiles), rope at evict, then k_nope/v from
     the block latent. k/v stay in SBUF; qT spills to DRAM (bf16).
  2. causal attention per (q-head, q-block): scores_T = kT.T @ qT,
     exp (|scores| bounded, no max-sub), diag-block masks, PV via
     v-stationary matmuls, DVE-accumulated denominators.
  3. partial o_proj in [hid, tok] layout, fp16 output.
"""

import numpy as np
import ml_dtypes

HIDDEN = 4096
N_HEADS = 32
KV_HEADS = 8
HEAD_DIM = 128
LOW_RANK = 64
TOP_K_ROPE = 32
ROPE_THETA = 10000.0
B, S = 2, 2048
NCORES = 8
TPG = 4                      # TP group size (cores per batch)
HPC = N_HEADS // TPG         # q heads per core = 8
KVPC = KV_HEADS // TPG       # kv heads per core = 2
QR = HPC * HEAD_DIM          # q rows per core = 1024
CD = LOW_RANK * KV_HEADS     # latent dim = 512
KRR = 64 * KVPC              # rope rows per core = 128

BF16 = ml_dtypes.bfloat16


def _rope_tables(seq_len):
    inv = 1.0 / (ROPE_THETA ** (np.arange(0, HEAD_DIM, 2, dtype=np.float32) / HEAD_DIM))
    pos = np.arange(seq_len, dtype=np.float32)
    fr = np.outer(pos, inv)
    emb = np.concatenate([fr, fr], axis=-1)          # [S, 128]
    return (np.cos(emb).T.astype(np.float32),        # [128, S]
            np.sin(emb).T.astype(np.float32))


def build_program(Sv=S, TB=512, QB=512):
    from concourse import bacc, tile, mybir
    import concourse.bass as bass

    f32 = mybir.dt.float32
    BF = mybir.dt.bfloat16
    F16 = mybir.dt.float16
    F32R = mybir.dt.float32r
    MS = bass.MemorySpace
    EXP = mybir.ActivationFunctionType.Exp

    NT = Sv                      # tokens per core (one batch)
    HT = HIDDEN // 128           # hidden tiles = 32
    NTB = NT // TB               # proj token blocks = 4
    NQB = NT // QB               # attention q blocks = 4
    NJ = QB // 128               # diagonal mask variants = 4
    QT = HPC                     # q-head tiles = 8
    LT = CD // 128               # latent tiles = 4
    NKT = NT // 128              # k tiles = 16

    nc = bacc.Bacc("TRN2", target_bir_lowering=False, debug=False,
                   num_devices=NCORES)

    def din(name, shape, dt=BF):
        return nc.dram_tensor(name, shape, dt, kind="ExternalInput").ap()

    hidT = din("hidT", [HIDDEN, NT])
    wq = din("wq_t", [HIDDEN, QR])
    wkd = din("wkd_t", [HIDDEN, CD + KRR])
    wupk = din("wupk_t", [CD, KRR])
    wupv = din("wupv_t", [CD, KVPC * HEAD_DIM])
    wo = din("wo_t", [QR, HIDDEN])
    qcos = din("qcos", [128, NT])
    qsin = din("qsin", [128, NT])
    kcos = din("kcos", [128, NT])
    ksin = din("ksin", [128, NT])
    masks = din("masks", [128, NJ, QB])
    onesd = din("ones", [128, 1])
    outp = nc.dram_tensor("out_part", [HIDDEN, NT], F16, kind="ExternalOutput").ap()
    qT_s = nc.dram_tensor("qT_s", [QT, 128, NT], BF).ap()

    with tile.TileContext(nc) as tc:
        with tc.tile_pool(name="persist", bufs=1) as pers:
            kT = pers.tile([128, KVPC, NT], BF, tag="kT")
            v_sb = pers.tile([128, NKT, KVPC, HEAD_DIM], BF, tag="v")

            # ---------------- phase 1: projections ----------------
            with tc.tile_pool(name="tabs", bufs=1) as tbp, \
                 tc.tile_pool(name="w1", bufs=1) as w1, \
                 tc.tile_pool(name="hb", bufs=1) as hbp, \
                 tc.tile_pool(name="hrt", bufs=6) as hrt, \
                 tc.tile_pool(name="cbp", bufs=2) as cbp, \
                 tc.tile_pool(name="st1", bufs=2) as st1, \
                 tc.tile_pool(name="ps1", bufs=8, space=MS.PSUM) as ps1:
                qcos_sb = tbp.tile([128, NT], BF, tag="qc")
                qsin_sb = tbp.tile([128, NT], BF, tag="qs")
                kcos_sb = tbp.tile([128, NT], BF, tag="kc")
                ksin_sb = tbp.tile([128, NT], BF, tag="ks")
                nc.sync.dma_start(qcos_sb[:], qcos)
                nc.sync.dma_start(qsin_sb[:], qsin)
                nc.sync.dma_start(kcos_sb[:], kcos)
                nc.sync.dma_start(ksin_sb[:], ksin)

                wq_sb = w1.tile([128, HT, QR], BF, tag="wq")
                wkd_sb = w1.tile([128, HT, CD + KRR], BF, tag="wkd")
                wupk_sb = w1.tile([128, LT, KRR], BF, tag="upk")
                wupv_sb = w1.tile([128, LT, KVPC * HEAD_DIM], BF, tag="upv")
                nc.sync.dma_start(wq_sb[:], wq.rearrange("(t p) w -> p t w", p=128))
                nc.sync.dma_start(wkd_sb[:], wkd.rearrange("(t p) w -> p t w", p=128))
                nc.sync.dma_start(wupk_sb[:], wupk.rearrange("(t p) w -> p t w", p=128))
                nc.sync.dma_start(wupv_sb[:], wupv.rearrange("(t p) w -> p t w", p=128))

                for blk in range(NTB):
                    c0, c1 = blk * TB, (blk + 1) * TB
                    # block buffer for the q sub-pass, DMA'd during the kd
                    # sub-pass (which streams hid again via a rotating pool)
                    hblk = hbp.tile([128, HT, TB], BF, tag="hid")
                    nc.sync.dma_start(
                        hblk[:], hidT[:, c0:c1].rearrange("(t p) w -> p t w", p=128))

                    # kd sub-pass: latent (4 tiles) + k-rope (1 tile)
                    dps = [ps1.tile([128, TB], f32, tag="ps", name=f"dps{_m}")
                           for _m in range(LT)]
                    krp = ps1.tile([128, TB], f32, tag="ps", name="krp")
                    for t in range(HT):
                        ht = hrt.tile([128, TB], BF, tag="ht")
                        nc.sync.dma_start(ht[:], hidT[t * 128:(t + 1) * 128, c0:c1])
                        for m in range(LT):
                            nc.tensor.matmul(
                                dps[m][:], wkd_sb[:, t, m * 128:(m + 1) * 128],
                                ht[:], start=(t == 0), stop=(t == HT - 1))
                        nc.tensor.matmul(
                            krp[:], wkd_sb[:, t, CD:CD + KRR],
                            ht[:], start=(t == 0), stop=(t == HT - 1))

                    cb = cbp.tile([128, LT, TB], BF, tag="cb")
                    for m in range(LT):
                        nc.scalar.copy(cb[:, m, :], dps[m][:])
                    # k-rope rows: per kv head 64 rows = [dims 0:32, dims 64:96]
                    rawk = st1.tile([128, TB], f32, tag="rawk")
                    nc.scalar.copy(rawk[:], krp[:])
                    rotk = st1.tile([128, TB], f32, tag="rotk")
                    nc.sync.dma_start(rotk[0:32, :], rawk[32:64, :])
                    nc.sync.dma_start(rotk[32:64, :], rawk[0:32, :])
                    nc.sync.dma_start(rotk[64:96, :], rawk[96:128, :])
                    nc.sync.dma_start(rotk[96:128, :], rawk[64:96, :])
                    nc.vector.tensor_mul(rawk[:], rawk[:], kcos_sb[:, c0:c1])
                    nc.vector.tensor_mul(rotk[:], rotk[:], ksin_sb[:, c0:c1])
                    for kv in range(KVPC):
                        r0 = kv * 64
                        nc.vector.tensor_add(kT[0:32, kv, c0:c1],
                                             rawk[r0:r0 + 32, :], rotk[r0:r0 + 32, :])
                        nc.vector.tensor_add(kT[64:96, kv, c0:c1],
                                             rawk[r0 + 32:r0 + 64, :], rotk[r0 + 32:r0 + 64, :])

                    # q sub-pass: 8 head tiles
                    qps = [ps1.tile([128, TB], f32, tag="ps", name=f"qps{_m}")
                           for _m in range(QT)]
                    for t in range(HT):
                        for m in range(QT):
                            nc.tensor.matmul(
                                qps[m][:], wq_sb[:, t, m * 128:(m + 1) * 128],
                                hblk[:, t, :], start=(t == 0), stop=(t == HT - 1))
                    for m in range(QT):
                        raw = st1.tile([128, TB], f32, tag="qraw")
                        nc.scalar.copy(raw[:], qps[m][:])
                        rot = st1.tile([128, TB], f32, tag="qrot")
                        nc.sync.dma_start(rot[0:64, :], raw[64:128, :])
                        nc.sync.dma_start(rot[64:128, :], raw[0:64, :])
                        nc.vector.tensor_mul(raw[:], raw[:], qcos_sb[:, c0:c1])
                        nc.vector.tensor_mul(rot[:], rot[:], qsin_sb[:, c0:c1])
                        qsb = st1.tile([128, TB], BF, tag="qsb")
                        nc.vector.tensor_add(qsb[:], raw[:], rot[:])
                        nc.sync.dma_start(qT_s[m, :, c0:c1], qsb[:])

                    # up-projections from the block latent
                    kcp = ps1.tile([128, TB], f32, tag="ps", name="kcp")
                    for lt in range(LT):
                        nc.tensor.matmul(kcp[:], wupk_sb[:, lt, :], cb[:, lt, :],
                                         start=(lt == 0), stop=(lt == LT - 1))
                    # rows: [kv0 d32:64, kv0 d96:128, kv1 d32:64, kv1 d96:128]
                    for kv in range(KVPC):
                        r0 = kv * 64
                        nc.scalar.copy(kT[32:64, kv, c0:c1], kcp[r0:r0 + 32, :])
                        nc.scalar.copy(kT[96:128, kv, c0:c1], kcp[r0 + 32:r0 + 64, :])
                    for tt in range(TB // 128):
                        vp = ps1.tile([128, TB], f32, tag="ps", name=f"vp{tt}")
                        for lt in range(LT):
                            nc.tensor.matmul(
                                vp[:, 0:KVPC * HEAD_DIM],
                                cb[:, lt, tt * 128:(tt + 1) * 128],
                                wupv_sb[:, lt, :],
                                start=(lt == 0), stop=(lt == LT - 1))
                        nc.scalar.copy(v_sb[:, blk * (TB // 128) + tt, :, :],
                                       vp[:, 0:KVPC * HEAD_DIM])

            # ---------------- phase 2: attention ----------------
            with tc.tile_pool(name="wop", bufs=1) as wop, \
                 tc.tile_pool(name="att", bufs=1) as ap_:
                wo_sb = wop.tile([128, QT, HIDDEN], BF, tag="wo")
                nc.sync.dma_start(wo_sb[:], wo.rearrange("(t p) w -> p t w", p=128))
                attn_sb = ap_.tile([128, QT, NT], BF, tag="attn")

                with tc.tile_pool(name="cst", bufs=1) as cst, \
                     tc.tile_pool(name="qh", bufs=2) as qhp, \
                     tc.tile_pool(name="pt", bufs=4) as ptp, \
                     tc.tile_pool(name="sm", bufs=2) as smp, \
                     tc.tile_pool(name="psS", bufs=3, space=MS.PSUM) as psS, \
                     tc.tile_pool(name="psO", bufs=2, space=MS.PSUM) as psO, \
                     tc.tile_pool(name="psU", bufs=2, space=MS.PSUM) as psU:
                    masks_sb = cst.tile([128, NJ, QB], BF, tag="masks")
                    nc.sync.dma_start(masks_sb[:], masks)
                    ones_sb = cst.tile([128, 1], BF, tag="ones")
                    nc.sync.dma_start(ones_sb[:], onesd)
                    for m in range(QT):
                        kvh = m // (HPC // KVPC)
                        qh = qhp.tile([128, NT], BF, tag="qh")
                        nc.sync.dma_start(qh[:], qT_s[m])
                        for qb in range(NQB):
                            nkt = (qb + 1) * NJ
                            ops = psO.tile([128, QB], f32, tag="ops")
                            sps = psU.tile([1, QB], f32, tag="sps")
                            for kt in range(nkt):
                                scp = psS.tile([128, QB], f32, tag="scp")
                                nc.tensor.matmul(
                                    scp[:],
                                    kT[:, kvh, kt * 128:(kt + 1) * 128],
                                    qh[:, qb * QB:(qb + 1) * QB],
                                    start=True, stop=True)
                                ptile = ptp.tile([128, QB], BF, tag="pt")
                                nc.scalar.activation(ptile[:], scp[:], EXP)
                                j = kt - qb * NJ
                                if j >= 0:
                                    nc.vector.tensor_mul(ptile[:], ptile[:],
                                                         masks_sb[:, j, :])
                                nc.tensor.matmul(
                                    ops[:], v_sb[:, kt, kvh, :], ptile[:],
                                    start=(kt == 0), stop=(kt == nkt - 1))
                                nc.tensor.matmul(
                                    sps[:], ones_sb[:], ptile[:],
                                    start=(kt == 0), stop=(kt == nkt - 1))
                            srow = smp.tile([1, QB], f32, tag="srow")
                            nc.scalar.copy(srow[:], sps[:])
                            sbc = smp.tile([128, QB], f32, tag="sbc")
                            nc.gpsimd.partition_broadcast(sbc[:], srow[:])
                            rbc = smp.tile([128, QB], f32, tag="rbc")
                            nc.vector.reciprocal(rbc[:], sbc[:])
                            nc.vector.tensor_mul(
                                attn_sb[:, m, qb * QB:(qb + 1) * QB],
                                ops[:], rbc[:])

                # ---------------- phase 3: partial o_proj ----------------
                with tc.tile_pool(name="st4", bufs=4) as st4, \
                     tc.tile_pool(name="ps4", bufs=8, space=MS.PSUM) as ps4:
                    for n in range(HT):
                        for j in range(NQB):
                            ps = ps4.tile([128, QB], f32, tag="ps")
                            for h2 in range(QT):
                                nc.tensor.matmul(
                                    ps[:],
                                    wo_sb[:, h2, n * 128:(n + 1) * 128],
                                    attn_sb[:, h2, j * QB:(j + 1) * QB],
                                    start=(h2 == 0), stop=(h2 == QT - 1))
                            osb = st4.tile([128, QB], F16, tag="osb")
                            nc.vector.tensor_copy(osb[:], ps[:])
                            nc.sync.dma_start(
                                outp[n * 128:(n + 1) * 128, j * QB:(j + 1) * QB],
                                osb[:])

    nc.compile()
    return nc


def make_in_maps(hidden_states, Wq, Wkr, Wdk, Wupk, Wupv, Wo, Sv=S, QB=512):
    """Host-side sharding + layout prep. Returns per-core input dicts."""
    NJ = QB // 128
    scale = 1.0 / np.sqrt(np.float32(HEAD_DIM))
    hidden_states = np.asarray(hidden_states, np.float32)
    Wq, Wkr, Wdk = np.asarray(Wq, np.float32), np.asarray(Wkr, np.float32), np.asarray(Wdk, np.float32)
    Wupk, Wupv, Wo = np.asarray(Wupk, np.float32), np.asarray(Wupv, np.float32), np.asarray(Wo, np.float32)

    cos_t, sin_t = _rope_tables(Sv)                    # [128, S]
    qcos = np.ascontiguousarray(cos_t * scale).astype(BF16)
    qsin = np.ascontiguousarray(
        np.concatenate([-sin_t[0:64], sin_t[64:128]], axis=0) * scale).astype(BF16)
    # per kv head 64 rope rows = [dims 0:32, dims 64:96], tiled x KVPC
    kc1 = np.concatenate([cos_t[0:32], cos_t[64:96]], axis=0)
    ks1 = np.concatenate([-sin_t[0:32], sin_t[64:96]], axis=0)
    kcos = np.ascontiguousarray(np.tile(kc1, (KVPC, 1))).astype(BF16)
    ksin = np.ascontiguousarray(np.tile(ks1, (KVPC, 1))).astype(BF16)

    k_idx = np.arange(128)[:, None]
    q_idx = np.arange(QB)[None, :]
    masks = np.stack(
        [(q_idx >= j * 128 + k_idx).astype(np.float32) for j in range(NJ)],
        axis=1)                                        # [128, NJ, QB]
    masks = np.ascontiguousarray(masks).astype(BF16)

    in_maps = []
    for c in range(NCORES):
        b, tp = c // TPG, c % TPG
        hidT = np.ascontiguousarray(hidden_states[b].T).astype(BF16)
        wq_t = np.ascontiguousarray(Wq[QR * tp:QR * (tp + 1)].T).astype(BF16)
        wkd_t = np.ascontiguousarray(
            np.concatenate([Wdk, Wkr[KRR * tp:KRR * (tp + 1)]], axis=0).T).astype(BF16)
        wupk_t = np.ascontiguousarray(Wupk[KRR * tp:KRR * (tp + 1)].T).astype(BF16)
        wupv_t = np.ascontiguousarray(
            Wupv[KVPC * HEAD_DIM * tp:KVPC * HEAD_DIM * (tp + 1)].T).astype(BF16)
        wo_t = np.ascontiguousarray(Wo[:, QR * tp:QR * (tp + 1)].T).astype(BF16)
        in_maps.append({
            "hidT": hidT, "wq_t": wq_t, "wkd_t": wkd_t,
            "wupk_t": wupk_t, "wupv_t": wupv_t, "wo_t": wo_t,
            "qcos": qcos, "qsin": qsin, "kcos": kcos, "ksin": ksin,
            "masks": masks, "ones": np.ones((128, 1), BF16),
        })
    return in_maps


def combine_outputs(res):
    outs = []
    for b in range(B):
        acc = res.results[b * TPG]["out_part"].astype(np.float32)
        for tp in range(1, TPG):
            acc = acc + res.results[b * TPG + tp]["out_part"].astype(np.float32)
        outs.append(acc.T)                             # [S, HIDDEN]
    return np.stack(outs).astype(np.float32)           # [B, S, HIDDEN]


_NC_CACHE = {}


def _get_program(key=(S, 512, 512)):
    if key not in _NC_CACHE:
        _NC_CACHE[key] = build_program(*key)
    return _NC_CACHE[key]


def kernel(hidden_states, Wq, Wkr, Wdk, Wupk, Wupv, Wo):
    from concourse.bass_utils import run_bass_kernel_spmd

    in_maps = make_in_maps(np.asarray(hidden_states), Wq, Wkr, Wdk, Wupk, Wupv, Wo)
    nc = _get_program()
    res = run_bass_kernel_spmd(nc, in_maps, list(range(NCORES)))
    return combine_outputs(res)


# revision 12
# speedup vs baseline: 1.4952x; 1.1835x over previous
"""MLA (CustomLlamaMLAForInfer) Trainium2 Bass kernel, v4.

Sharding: batch x tensor-parallel. Core c owns batch c//4 and TP shard
tp=c%4: 8 q-heads, 2 kv-heads, 1024 Wo columns, and a 128-row shard of
the shared latent projection Wdk. Latent shards are AllGathered within
each batch group of 4 on device. o_proj partials ([HIDDEN, 2048] fp16,
transposed layout) are summed per batch group on the host.

All matmul operands are fp16 (fp32 PSUM accumulation); rope math runs
in fp32 on the DVE at PSUM-evict time. Softmax denominators accumulate
on the DVE in fp16 (scores are bounded, exp stays in fp16 range) and
are reduced across partitions with one ones-matmul per (head, q-block).

Device phases (single SPMD program; per-core weights differ):
  1. per 512-token block: latent-shard + k-rope projections (2 PSUM
     tiles), q projection (8 PSUM tiles, head-outer so evictions
     pipeline), rope at evict. qT spills to DRAM fp16.
  1g. AllGather latent shards -> full 512-dim latent (DRAM, fp16).
  1u. per block: k_nope/v up-projections from the gathered latent.
  2. causal attention per (q-head, q-block): paired score tiles, one
     exp per pair, diag masks, PV with v-stationary matmuls,
     fp16 DVE denominator accumulation, approx reciprocal.
  3. partial o_proj in [hid, tok] layout, fp16 output.
"""

import numpy as np

HIDDEN = 4096
N_HEADS = 32
KV_HEADS = 8
HEAD_DIM = 128
LOW_RANK = 64
TOP_K_ROPE = 32
ROPE_THETA = 10000.0
B, S = 2, 2048
NCORES = 8
TPG = 4                      # TP group size (cores per batch)
HPC = N_HEADS // TPG         # q heads per core = 8
KVPC = KV_HEADS // TPG       # kv heads per core = 2
QR = HPC * HEAD_DIM          # q rows per core = 1024
CD = LOW_RANK * KV_HEADS     # latent dim = 512
CDS = CD // TPG              # latent shard rows per core = 128
KRR = 64 * KVPC              # rope rows per core = 128
F16 = np.float16


def _rope_tables(seq_len):
    inv = 1.0 / (ROPE_THETA ** (np.arange(0, HEAD_DIM, 2, dtype=np.float32) / HEAD_DIM))
    pos = np.arange(seq_len, dtype=np.float32)
    fr = np.outer(pos, inv)
    emb = np.concatenate([fr, fr], axis=-1)          # [S, 128]
    return (np.cos(emb).T.astype(np.float32),        # [128, S]
            np.sin(emb).T.astype(np.float32))


def build_program(Sv=S, TB=512, QB=512):
    from concourse import bacc, tile, mybir
    import concourse.bass as bass

    f32 = mybir.dt.float32
    FP = mybir.dt.float16
    MS = bass.MemorySpace
    EXP = mybir.ActivationFunctionType.Exp

    NT = Sv                      # tokens per core (one batch)
    HT = HIDDEN // 128           # hidden tiles = 32
    NTB = NT // TB               # proj token blocks = 4
    NQB = NT // QB               # attention q blocks = 4
    NJ = QB // 128               # diagonal mask variants = 4
    QT = HPC                     # q-head tiles = 8
    LT = CD // 128               # latent tiles = 4
    NKT = NT // 128              # k tiles = 16

    nc = bacc.Bacc("TRN2", target_bir_lowering=False, debug=False,
                   num_devices=NCORES)

    def din(name, shape, dt=FP):
        return nc.dram_tensor(name, shape, dt, kind="ExternalInput").ap()

    hidT = din("hidT", [HIDDEN, NT])
    wq = din("wq_t", [HIDDEN, QR])
    wkd = din("wkd_t", [HIDDEN, CDS + KRR])
    wupk = din("wupk_t", [CD, KRR])
    wupv = din("wupv_t", [CD, KVPC * HEAD_DIM])
    wo = din("wo_t", [QR, HIDDEN])
    qcos = din("qcos", [128, NT])
    qsin = din("qsin", [128, NT])
    kcos = din("kcos", [128, NT])
    ksin = din("ksin", [128, NT])
    masks = din("masks", [128, NJ, QB])
    onesd = din("ones", [128, 1])
    outp = nc.dram_tensor("out_part", [HIDDEN, NT], FP, kind="ExternalOutput").ap()
    qT_s = nc.dram_tensor("qT_s", [QT, 128, NT], FP).ap()
    ckv_sh = nc.dram_tensor("ckv_sh", [128, NT], FP).ap()
    ckv_g = nc.dram_tensor("ckv_g", [TPG, 128, NT], FP).ap()
    cc_groups = [[g * TPG + i for i in range(TPG)] for g in range(NCORES // TPG)]

    with tile.TileContext(nc) as tc:
        with tc.tile_pool(name="persist", bufs=1) as pers:
            kT = pers.tile([128, KVPC, NT], FP, tag="kT")
            v_sb = pers.tile([128, NKT, KVPC, HEAD_DIM], FP, tag="v")

            # ---------------- phase 1: hid projections ----------------
            with tc.tile_pool(name="tabs", bufs=1) as tbp, \
                 tc.tile_pool(name="w1", bufs=1) as w1, \
                 tc.tile_pool(name="hb", bufs=2) as hbp, \
                 tc.tile_pool(name="cbp", bufs=2) as cbp, \
                 tc.tile_pool(name="st1", bufs=2) as st1, \
                 tc.tile_pool(name="ps1", bufs=8, space=MS.PSUM) as ps1:
                # DMA issue order matters for the cold start: the first kd
                # sub-pass needs only wkd + the first hid block.
                wq_sb = w1.tile([128, HT, QR], FP, tag="wq")
                wkd_sb = w1.tile([128, HT, CDS + KRR], FP, tag="wkd")
                wupk_sb = w1.tile([128, LT, KRR], FP, tag="upk")
                wupv_sb = w1.tile([128, LT, KVPC * HEAD_DIM], FP, tag="upv")
                nc.sync.dma_start(wkd_sb[:], wkd.rearrange("(t p) w -> p t w", p=128))
                hblks = [hbp.tile([128, HT, TB], FP, tag="hid", name=f"hb{_b}")
                         for _b in range(NTB)]
                nc.sync.dma_start(
                    hblks[0][:], hidT[:, 0:TB].rearrange("(t p) w -> p t w", p=128))
                nc.sync.dma_start(wq_sb[:, :, 0:QR // 2],
                                  wq[:, 0:QR // 2].rearrange("(t p) w -> p t w", p=128))
                nc.sync.dma_start(wq_sb[:, :, QR // 2:],
                                  wq[:, QR // 2:].rearrange("(t p) w -> p t w", p=128))
                qcos_sb = tbp.tile([128, NT], FP, tag="qc")
                qsin_sb = tbp.tile([128, NT], FP, tag="qs")
                kcos_sb = tbp.tile([128, NT], FP, tag="kc")
                ksin_sb = tbp.tile([128, NT], FP, tag="ks")
                nc.sync.dma_start(kcos_sb[:], kcos)
                nc.sync.dma_start(ksin_sb[:], ksin)
                nc.sync.dma_start(qcos_sb[:], qcos)
                nc.sync.dma_start(qsin_sb[:], qsin)
                nc.sync.dma_start(wupk_sb[:], wupk.rearrange("(t p) w -> p t w", p=128))
                nc.sync.dma_start(wupv_sb[:], wupv.rearrange("(t p) w -> p t w", p=128))

                def emit_kd(blk):
                    c0, c1 = blk * TB, (blk + 1) * TB
                    hblk = hblks[blk]
                    # kd sub-pass: latent shard (1 tile) + k-rope (1 tile)
                    csp = ps1.tile([128, TB], f32, tag="ps", name="csp")
                    krp = ps1.tile([128, TB], f32, tag="ps", name="krp")
                    for t in range(HT):
                        nc.tensor.matmul(
                            csp[:], wkd_sb[:, t, 0:CDS], hblk[:, t, :],
                            start=(t == 0), stop=(t == HT - 1))
                        nc.tensor.matmul(
                            krp[:], wkd_sb[:, t, CDS:CDS + KRR], hblk[:, t, :],
                            start=(t == 0), stop=(t == HT - 1))
                    cst = st1.tile([128, TB], FP, tag="cst")
                    nc.scalar.copy(cst[:], csp[:])
                    nc.sync.dma_start(ckv_sh[:, c0:c1], cst[:])
                    # k-rope rows: per kv head 64 rows = [dims 0:32, dims 64:96]
                    rawk = st1.tile([128, TB], f32, tag="rawk")
                    nc.scalar.copy(rawk[:], krp[:])
                    rotk = st1.tile([128, TB], f32, tag="rotk")
                    nc.sync.dma_start(rotk[0:32, :], rawk[32:64, :])
                    nc.sync.dma_start(rotk[32:64, :], rawk[0:32, :])
                    nc.sync.dma_start(rotk[64:96, :], rawk[96:128, :])
                    nc.sync.dma_start(rotk[96:128, :], rawk[64:96, :])
                    nc.vector.tensor_mul(rawk[:], rawk[:], kcos_sb[:, c0:c1])
                    nc.vector.tensor_mul(rotk[:], rotk[:], ksin_sb[:, c0:c1])
                    for kv in range(KVPC):
                        r0 = kv * 64
                        nc.vector.tensor_add(kT[0:32, kv, c0:c1],
                                             rawk[r0:r0 + 32, :], rotk[r0:r0 + 32, :])
                        nc.vector.tensor_add(kT[64:96, kv, c0:c1],
                                             rawk[r0 + 32:r0 + 64, :], rotk[r0 + 32:r0 + 64, :])

                def emit_q(blk, m):
                    c0, c1 = blk * TB, (blk + 1) * TB
                    hblk = hblks[blk]
                    qp = ps1.tile([128, TB], f32, tag="ps", name=f"qp{m}")
                    for t in range(HT):
                        nc.tensor.matmul(
                            qp[:], wq_sb[:, t, m * 128:(m + 1) * 128],
                            hblk[:, t, :], start=(t == 0), stop=(t == HT - 1))
                    raw = st1.tile([128, TB], f32, tag="qraw")
                    nc.scalar.copy(raw[:], qp[:])
                    rot = st1.tile([128, TB], f32, tag="qrot")
                    nc.sync.dma_start(rot[0:64, :], raw[64:128, :])
                    nc.sync.dma_start(rot[64:128, :], raw[0:64, :])
                    nc.vector.tensor_mul(raw[:], raw[:], qcos_sb[:, c0:c1])
                    nc.vector.tensor_mul(rot[:], rot[:], qsin_sb[:, c0:c1])
                    qsb = st1.tile([128, TB], FP, tag="qsb")
                    nc.vector.tensor_add(qsb[:], raw[:], rot[:])
                    nc.sync.dma_start(qT_s[m, :, c0:c1], qsb[:])

                def emit_up(blk):
                    c0, c1 = blk * TB, (blk + 1) * TB
                    cb = cbp.tile([128, LT, TB], FP, tag="cb")
                    nc.sync.dma_start(
                        cb[:], ckv_g[:, :, c0:c1].rearrange("g p w -> p g w"))
                    kcp = ps1.tile([128, TB], f32, tag="ps", name="kcp")
                    for lt in range(LT):
                        nc.tensor.matmul(kcp[:], wupk_sb[:, lt, :], cb[:, lt, :],
                                         start=(lt == 0), stop=(lt == LT - 1))
                    # rows: [kv0 d32:64, kv0 d96:128, kv1 d32:64, kv1 d96:128]
                    for kv in range(KVPC):
                        r0 = kv * 64
                        nc.scalar.copy(kT[32:64, kv, c0:c1], kcp[r0:r0 + 32, :])
                        nc.scalar.copy(kT[96:128, kv, c0:c1], kcp[r0 + 32:r0 + 64, :])
                    for tt in range(TB // 128):
                        vp = ps1.tile([128, TB], f32, tag="ps", name=f"vp{tt}")
                        for lt in range(LT):
                            nc.tensor.matmul(
                                vp[:, 0:KVPC * HEAD_DIM],
                                cb[:, lt, tt * 128:(tt + 1) * 128],
                                wupv_sb[:, lt, :],
                                start=(lt == 0), stop=(lt == LT - 1))
                        nc.scalar.copy(v_sb[:, blk * (TB // 128) + tt, :, :],
                                       vp[:, 0:KVPC * HEAD_DIM])

                for blk in range(NTB):
                    if blk + 1 < NTB:
                        c0n = (blk + 1) * TB
                        nc.sync.dma_start(
                            hblks[blk + 1][:],
                            hidT[:, c0n:c0n + TB].rearrange("(t p) w -> p t w", p=128))
                    emit_kd(blk)
                    if blk == NTB - 1:
                        # all latent shards written; gather runs during the
                        # final q sub-pass, up-projections interleave after it
                        nc.gpsimd.collective_compute(
                            "AllGather", mybir.AluOpType.bypass, cc_groups,
                            ins=[ckv_sh], outs=[ckv_g])
                    for m in range(QT):
                        emit_q(blk, m)
                        if blk == NTB - 1 and m >= QT - NTB:
                            emit_up(m - (QT - NTB))

            # ---------------- phase 2: attention ----------------
            with tc.tile_pool(name="wop", bufs=1) as wop, \
                 tc.tile_pool(name="att", bufs=1) as ap_:
                wo_sb = wop.tile([128, QT, HIDDEN], FP, tag="wo")
                nc.sync.dma_start(wo_sb[:], wo.rearrange("(t p) w -> p t w", p=128))
                attn_sb = ap_.tile([128, QT, NT], FP, tag="attn")

                with tc.tile_pool(name="cst", bufs=1) as cst2, \
                     tc.tile_pool(name="qh", bufs=2) as qhp, \
                     tc.tile_pool(name="pt", bufs=3) as ptp, \
                     tc.tile_pool(name="sac", bufs=2) as sap, \
                     tc.tile_pool(name="sm", bufs=2) as smp, \
                     tc.tile_pool(name="psS", bufs=2, space=MS.PSUM) as psS, \
                     tc.tile_pool(name="psO", bufs=2, space=MS.PSUM) as psO, \
                     tc.tile_pool(name="psU", bufs=2, space=MS.PSUM) as psU:
                    masks_sb = cst2.tile([128, NJ, QB], FP, tag="masks")
                    nc.sync.dma_start(masks_sb[:], masks)
                    ones_sb = cst2.tile([128, 1], FP, tag="ones")
                    nc.sync.dma_start(ones_sb[:], onesd)
                    for m in range(QT):
                        kvh = m // (HPC // KVPC)
                        qh = qhp.tile([128, NT], FP, tag="qh")
                        nc.sync.dma_start(qh[:], qT_s[m])
                        for qb in range(NQB):
                            nkt = (qb + 1) * NJ
                            ops = psO.tile([128, QB], f32, tag="ops")
                            sacc = sap.tile([128, QB], FP, tag="sacc")
                            for p2 in range(nkt // 2):
                                scp = psS.tile([128, 2, QB], f32, tag="scp")
                                for h in range(2):
                                    kt = 2 * p2 + h
                                    nc.tensor.matmul(
                                        scp[:, h, :],
                                        kT[:, kvh, kt * 128:(kt + 1) * 128],
                                        qh[:, qb * QB:(qb + 1) * QB],
                                        start=True, stop=True)
                                pt = ptp.tile([128, 2, QB], FP, tag="pt")
                                nc.scalar.activation(pt[:], scp[:], EXP)
                                jj = 2 * p2 - qb * NJ
                                if jj >= 0:
                                    nc.vector.tensor_mul(pt[:], pt[:],
                                                         masks_sb[:, jj:jj + 2, :])
                                for h in range(2):
                                    kt = 2 * p2 + h
                                    nc.tensor.matmul(
                                        ops[:], v_sb[:, kt, kvh, :], pt[:, h, :],
                                        start=(kt == 0), stop=(kt == nkt - 1))
                                if p2 == 0:
                                    nc.vector.tensor_copy(sacc[:], pt[:, 0, :])
                                else:
                                    nc.vector.tensor_add(sacc[:], sacc[:], pt[:, 0, :])
                                nc.vector.tensor_add(sacc[:], sacc[:], pt[:, 1, :])
                            sps = psU.tile([1, QB], f32, tag="sps")
                            nc.tensor.matmul(sps[:], ones_sb[:], sacc[:],
                                             start=True, stop=True)
                            srow = smp.tile([1, QB], f32, tag="srow")
                            nc.scalar.copy(srow[:], sps[:])
                            sbc = smp.tile([128, QB], f32, tag="sbc")
                            nc.gpsimd.partition_broadcast(sbc[:], srow[:])
                            rbc = smp.tile([128, QB], f32, tag="rbc")
                            nc.vector.reciprocal_approx_fast(rbc[:], sbc[:])
                            nc.vector.tensor_mul(
                                attn_sb[:, m, qb * QB:(qb + 1) * QB],
                                ops[:], rbc[:])

                # ---------------- phase 3: partial o_proj ----------------
                with tc.tile_pool(name="st4", bufs=4) as st4, \
                     tc.tile_pool(name="ps4", bufs=8, space=MS.PSUM) as ps4:
                    for n in range(HT):
                        for j in range(NQB):
                            ps = ps4.tile([128, QB], f32, tag="ps")
                            for h2 in range(QT):
                                nc.tensor.matmul(
                                    ps[:],
                                    wo_sb[:, h2, n * 128:(n + 1) * 128],
                                    attn_sb[:, h2, j * QB:(j + 1) * QB],
                                    start=(h2 == 0), stop=(h2 == QT - 1))
                            osb = st4.tile([128, QB], FP, tag="osb")
                            nc.vector.tensor_copy(osb[:], ps[:])
                            nc.sync.dma_start(
                                outp[n * 128:(n + 1) * 128, j * QB:(j + 1) * QB],
                                osb[:])

    nc.compile()
    return nc


def make_in_maps(hidden_states, Wq, Wkr, Wdk, Wupk, Wupv, Wo, Sv=S, QB=512):
    """Host-side sharding + layout prep. Returns per-core input dicts."""
    NJ = QB // 128
    scale = 1.0 / np.sqrt(np.float32(HEAD_DIM))
    hidden_states = np.asarray(hidden_states, np.float32)
    Wq, Wkr, Wdk = np.asarray(Wq, np.float32), np.asarray(Wkr, np.float32), np.asarray(Wdk, np.float32)
    Wupk, Wupv, Wo = np.asarray(Wupk, np.float32), np.asarray(Wupv, np.float32), np.asarray(Wo, np.float32)

    cos_t, sin_t = _rope_tables(Sv)                    # [128, S]
    qcos = np.ascontiguousarray(cos_t * scale).astype(F16)
    qsin = np.ascontiguousarray(
        np.concatenate([-sin_t[0:64], sin_t[64:128]], axis=0) * scale).astype(F16)
    # per kv head 64 rope rows = [dims 0:32, dims 64:96], tiled x KVPC
    kc1 = np.concatenate([cos_t[0:32], cos_t[64:96]], axis=0)
    ks1 = np.concatenate([-sin_t[0:32], sin_t[64:96]], axis=0)
    kcos = np.ascontiguousarray(np.tile(kc1, (KVPC, 1))).astype(F16)
    ksin = np.ascontiguousarray(np.tile(ks1, (KVPC, 1))).astype(F16)

    k_idx = np.arange(128)[:, None]
    q_idx = np.arange(QB)[None, :]
    masks = np.stack(
        [(q_idx >= j * 128 + k_idx).astype(np.float32) for j in range(NJ)],
        axis=1)                                        # [128, NJ, QB]
    masks = np.ascontiguousarray(masks).astype(F16)

    in_maps = []
    for c in range(NCORES):
        b, tp = c // TPG, c % TPG
        hidT = np.ascontiguousarray(hidden_states[b].T).astype(F16)
        wq_t = np.ascontiguousarray(Wq[QR * tp:QR * (tp + 1)].T).astype(F16)
        wkd_t = np.ascontiguousarray(
            np.concatenate([Wdk[CDS * tp:CDS * (tp + 1)],
                            Wkr[KRR * tp:KRR * (tp + 1)]], axis=0).T).astype(F16)
        wupk_t = np.ascontiguousarray(Wupk[KRR * tp:KRR * (tp + 1)].T).astype(F16)
        wupv_t = np.ascontiguousarray(
            Wupv[KVPC * HEAD_DIM * tp:KVPC * HEAD_DIM * (tp + 1)].T).astype(F16)
        wo_t = np.ascontiguousarray(Wo[:, QR * tp:QR * (tp + 1)].T).astype(F16)
        in_maps.append({
            "hidT": hidT, "wq_t": wq_t, "wkd_t": wkd_t,
            "wupk_t": wupk_t, "wupv_t": wupv_t, "wo_t": wo_t,
            "qcos": qcos, "qsin": qsin, "kcos": kcos, "ksin": ksin,
            "masks": masks, "ones": np.ones((128, 1), F16),
        })
    return in_maps


def combine_outputs(res):
    outs = []
    for b in range(B):
        acc = res.results[b * TPG]["out_part"].astype(np.float32)
        for tp in range(1, TPG):
            acc = acc + res.results[b * TPG + tp]["out_part"].astype(np.float32)
        outs.append(acc.T)                             # [S, HIDDEN]
    return np.stack(outs).astype(np.float32)           # [B, S, HIDDEN]


_NC_CACHE = {}


def _get_program(key=(S, 512, 512)):
    if key not in _NC_CACHE:
        _NC_CACHE[key] = build_program(*key)
    return _NC_CACHE[key]


def kernel(hidden_states, Wq, Wkr, Wdk, Wupk, Wupv, Wo):
    from concourse.bass_utils import run_bass_kernel_spmd

    in_maps = make_in_maps(np.asarray(hidden_states), Wq, Wkr, Wdk, Wupk, Wupv, Wo)
    nc = _get_program()
    res = run_bass_kernel_spmd(nc, in_maps, list(range(NCORES)))
    return combine_outputs(res)


# revision 19
# speedup vs baseline: 1.5223x; 1.0181x over previous
"""MLA (CustomLlamaMLAForInfer) Trainium2 Bass kernel, v4.

Sharding: batch x tensor-parallel. Core c owns batch c//4 and TP shard
tp=c%4: 8 q-heads, 2 kv-heads, 1024 Wo columns, and a 128-row shard of
the shared latent projection Wdk. Latent shards are AllGathered within
each batch group of 4 on device. o_proj partials ([HIDDEN, 2048] fp16,
transposed layout) are summed per batch group on the host.

All matmul operands are fp16 (fp32 PSUM accumulation); rope math runs
in fp32 on the DVE at PSUM-evict time. Softmax denominators accumulate
on the DVE in fp16 (scores are bounded, exp stays in fp16 range) and
are reduced across partitions with one ones-matmul per (head, q-block).

Device phases (single SPMD program; per-core weights differ):
  1. per 512-token block: latent-shard + k-rope projections (2 PSUM
     tiles), q projection (8 PSUM tiles, head-outer so evictions
     pipeline), rope at evict. qT spills to DRAM fp16.
  1g. AllGather latent shards -> full 512-dim latent (DRAM, fp16).
  1u. per block: k_nope/v up-projections from the gathered latent.
  2. causal attention per (q-head, q-block): paired score tiles, one
     exp per pair, diag masks, PV with v-stationary matmuls,
     fp16 DVE denominator accumulation, approx reciprocal.
  3. partial o_proj in [hid, tok] layout, fp16 output.
"""

import numpy as np

HIDDEN = 4096
N_HEADS = 32
KV_HEADS = 8
HEAD_DIM = 128
LOW_RANK = 64
TOP_K_ROPE = 32
ROPE_THETA = 10000.0
B, S = 2, 2048
NCORES = 8
TPG = 4                      # TP group size (cores per batch)
HPC = N_HEADS // TPG         # q heads per core = 8
KVPC = KV_HEADS // TPG       # kv heads per core = 2
QR = HPC * HEAD_DIM          # q rows per core = 1024
CD = LOW_RANK * KV_HEADS     # latent dim = 512
CDS = CD // TPG              # latent shard rows per core = 128
KRR = 64 * KVPC              # rope rows per core = 128
F16 = np.float16


def _rope_tables(seq_len):
    inv = 1.0 / (ROPE_THETA ** (np.arange(0, HEAD_DIM, 2, dtype=np.float32) / HEAD_DIM))
    pos = np.arange(seq_len, dtype=np.float32)
    fr = np.outer(pos, inv)
    emb = np.concatenate([fr, fr], axis=-1)          # [S, 128]
    return (np.cos(emb).T.astype(np.float32),        # [128, S]
            np.sin(emb).T.astype(np.float32))


def build_program(Sv=S, TB=512, QB=512):
    from concourse import bacc, tile, mybir
    import concourse.bass as bass

    f32 = mybir.dt.float32
    FP = mybir.dt.float16
    MS = bass.MemorySpace
    EXP = mybir.ActivationFunctionType.Exp

    NT = Sv                      # tokens per core (one batch)
    HT = HIDDEN // 128           # hidden tiles = 32
    NTB = NT // TB               # proj token blocks = 4
    NQB = NT // QB               # attention q blocks = 4
    NJ = QB // 128               # diagonal mask variants = 4
    QT = HPC                     # q-head tiles = 8
    LT = CD // 128               # latent tiles = 4
    NKT = NT // 128              # k tiles = 16

    nc = bacc.Bacc("TRN2", target_bir_lowering=False, debug=False,
                   num_devices=NCORES)

    def din(name, shape, dt=FP):
        return nc.dram_tensor(name, shape, dt, kind="ExternalInput").ap()

    hidT = din("hidT", [HIDDEN, NT])
    wq = din("wq_t", [HIDDEN, QR])
    wkd = din("wkd_t", [HIDDEN, CDS + KRR])
    wupk = din("wupk_t", [CD, KRR])
    wupv = din("wupv_t", [CD, KVPC * HEAD_DIM])
    wo = din("wo_t", [QR, HIDDEN])
    qcos = din("qcos", [128, NT])
    qsin = din("qsin", [128, NT])
    kcos = din("kcos", [128, NT])
    ksin = din("ksin", [128, NT])
    masks = din("masks", [128, NJ, QB])
    onesd = din("ones", [128, 1])
    outp = nc.dram_tensor("out_part", [HIDDEN, NT], FP, kind="ExternalOutput").ap()
    qT_s = nc.dram_tensor("qT_s", [QT, 128, NT], FP).ap()
    NH = NT // 2
    ckv_shA = nc.dram_tensor("ckv_shA", [128, NH], FP).ap()
    ckv_shB = nc.dram_tensor("ckv_shB", [128, NH], FP).ap()
    ckv_gA = nc.dram_tensor("ckv_gA", [TPG, 128, NH], FP).ap()
    ckv_gB = nc.dram_tensor("ckv_gB", [TPG, 128, NH], FP).ap()
    cc_groups = [[g * TPG + i for i in range(TPG)] for g in range(NCORES // TPG)]

    with tile.TileContext(nc) as tc:
        with tc.tile_pool(name="persist", bufs=1) as pers:
            kT = pers.tile([128, KVPC, NT], FP, tag="kT")
            v_sb = pers.tile([128, NKT, KVPC, HEAD_DIM], FP, tag="v")

            # ---------------- phase 1: hid projections ----------------
            with tc.tile_pool(name="tabs", bufs=1) as tbp, \
                 tc.tile_pool(name="w1", bufs=1) as w1, \
                 tc.tile_pool(name="hb", bufs=2) as hbp, \
                 tc.tile_pool(name="cbp", bufs=2) as cbp, \
                 tc.tile_pool(name="st1", bufs=2) as st1, \
                 tc.tile_pool(name="ps1", bufs=8, space=MS.PSUM) as ps1:
                # DMA issue order matters for the cold start: the first kd
                # sub-pass needs only wkd + the first hid block.
                wq_sb = w1.tile([128, HT, QR], FP, tag="wq")
                wkd_sb = w1.tile([128, HT, CDS + KRR], FP, tag="wkd")
                wupk_sb = w1.tile([128, LT, KRR], FP, tag="upk")
                wupv_sb = w1.tile([128, LT, KVPC * HEAD_DIM], FP, tag="upv")
                nc.sync.dma_start(
                    wkd_sb[:, 0:HT // 2, :],
                    wkd[0:HIDDEN // 2].rearrange("(t p) w -> p t w", p=128))
                hblks = [hbp.tile([128, HT, TB], FP, tag="hid", name=f"hb{_b}")
                         for _b in range(NTB)]
                nc.sync.dma_start(
                    hblks[0][:, 0:HT // 2, :],
                    hidT[0:HIDDEN // 2, 0:TB].rearrange("(t p) w -> p t w", p=128))
                nc.sync.dma_start(
                    wkd_sb[:, HT // 2:, :],
                    wkd[HIDDEN // 2:].rearrange("(t p) w -> p t w", p=128))
                nc.sync.dma_start(
                    hblks[0][:, HT // 2:, :],
                    hidT[HIDDEN // 2:, 0:TB].rearrange("(t p) w -> p t w", p=128))
                nc.sync.dma_start(wq_sb[:, :, 0:QR // 2],
                                  wq[:, 0:QR // 2].rearrange("(t p) w -> p t w", p=128))
                nc.sync.dma_start(wq_sb[:, :, QR // 2:],
                                  wq[:, QR // 2:].rearrange("(t p) w -> p t w", p=128))
                qcos_sb = tbp.tile([128, NT], FP, tag="qc")
                qsin_sb = tbp.tile([128, NT], FP, tag="qs")
                kcos_sb = tbp.tile([128, NT], FP, tag="kc")
                ksin_sb = tbp.tile([128, NT], FP, tag="ks")
                nc.sync.dma_start(kcos_sb[:], kcos)
                nc.sync.dma_start(ksin_sb[:], ksin)
                nc.sync.dma_start(qcos_sb[:], qcos)
                nc.sync.dma_start(qsin_sb[:], qsin)
                nc.sync.dma_start(wupk_sb[:], wupk.rearrange("(t p) w -> p t w", p=128))
                nc.sync.dma_start(wupv_sb[:], wupv.rearrange("(t p) w -> p t w", p=128))

                def emit_kd(blk):
                    c0, c1 = blk * TB, (blk + 1) * TB
                    hblk = hblks[blk]
                    # kd sub-pass: latent shard (1 tile) + k-rope (1 tile)
                    csp = ps1.tile([128, TB], f32, tag="ps", name="csp")
                    krp = ps1.tile([128, TB], f32, tag="ps", name="krp")
                    for t in range(HT):
                        nc.tensor.matmul(
                            csp[:], wkd_sb[:, t, 0:CDS], hblk[:, t, :],
                            start=(t == 0), stop=(t == HT - 1))
                        nc.tensor.matmul(
                            krp[:], wkd_sb[:, t, CDS:CDS + KRR], hblk[:, t, :],
                            start=(t == 0), stop=(t == HT - 1))
                    cst = st1.tile([128, TB], FP, tag="cst")
                    nc.scalar.copy(cst[:], csp[:])
                    sh, s0 = (ckv_shA, c0) if blk < NTB // 2 else (ckv_shB, c0 - NH)
                    nc.sync.dma_start(sh[:, s0:s0 + TB], cst[:])
                    # k-rope rows: per kv head 64 rows = [dims 0:32, dims 64:96]
                    rawk = st1.tile([128, TB], f32, tag="rawk")
                    nc.scalar.copy(rawk[:], krp[:])
                    rotk = st1.tile([128, TB], f32, tag="rotk")
                    nc.sync.dma_start(rotk[0:32, :], rawk[32:64, :])
                    nc.sync.dma_start(rotk[32:64, :], rawk[0:32, :])
                    nc.sync.dma_start(rotk[64:96, :], rawk[96:128, :])
                    nc.sync.dma_start(rotk[96:128, :], rawk[64:96, :])
                    nc.vector.tensor_mul(rawk[:], rawk[:], kcos_sb[:, c0:c1])
                    nc.vector.tensor_mul(rotk[:], rotk[:], ksin_sb[:, c0:c1])
                    for kv in range(KVPC):
                        r0 = kv * 64
                        nc.vector.tensor_add(kT[0:32, kv, c0:c1],
                                             rawk[r0:r0 + 32, :], rotk[r0:r0 + 32, :])
                        nc.vector.tensor_add(kT[64:96, kv, c0:c1],
                                             rawk[r0 + 32:r0 + 64, :], rotk[r0 + 32:r0 + 64, :])

                def emit_q(blk, m):
                    c0, c1 = blk * TB, (blk + 1) * TB
                    hblk = hblks[blk]
                    qp = ps1.tile([128, TB], f32, tag="ps", name=f"qp{m}")
                    for t in range(HT):
                        nc.tensor.matmul(
                            qp[:], wq_sb[:, t, m * 128:(m + 1) * 128],
                            hblk[:, t, :], start=(t == 0), stop=(t == HT - 1))
                    raw = st1.tile([128, TB], f32, tag="qraw")
                    nc.scalar.copy(raw[:], qp[:])
                    rot = st1.tile([128, TB], f32, tag="qrot")
                    nc.sync.dma_start(rot[0:64, :], raw[64:128, :])
                    nc.sync.dma_start(rot[64:128, :], raw[0:64, :])
                    nc.vector.tensor_mul(raw[:], raw[:], qcos_sb[:, c0:c1])
                    nc.vector.tensor_mul(rot[:], rot[:], qsin_sb[:, c0:c1])
                    qsb = st1.tile([128, TB], FP, tag="qsb")
                    nc.vector.tensor_add(qsb[:], raw[:], rot[:])
                    nc.sync.dma_start(qT_s[m, :, c0:c1], qsb[:])

                def emit_up(blk):
                    c0, c1 = blk * TB, (blk + 1) * TB
                    g, g0 = (ckv_gA, c0) if blk < NTB // 2 else (ckv_gB, c0 - NH)
                    cb = cbp.tile([128, LT, TB], FP, tag="cb")
                    nc.sync.dma_start(
                        cb[:], g[:, :, g0:g0 + TB].rearrange("g p w -> p g w"))
                    kcp = ps1.tile([128, TB], f32, tag="ps", name="kcp")
                    for lt in range(LT):
                        nc.tensor.matmul(kcp[:], wupk_sb[:, lt, :], cb[:, lt, :],
                                         start=(lt == 0), stop=(lt == LT - 1))
                    # rows: [kv0 d32:64, kv0 d96:128, kv1 d32:64, kv1 d96:128]
                    for kv in range(KVPC):
                        r0 = kv * 64
                        nc.scalar.copy(kT[32:64, kv, c0:c1], kcp[r0:r0 + 32, :])
                        nc.scalar.copy(kT[96:128, kv, c0:c1], kcp[r0 + 32:r0 + 64, :])
                    for tt in range(TB // 128):
                        vp = ps1.tile([128, TB], f32, tag="ps", name=f"vp{tt}")
                        for lt in range(LT):
                            nc.tensor.matmul(
                                vp[:, 0:KVPC * HEAD_DIM],
                                cb[:, lt, tt * 128:(tt + 1) * 128],
                                wupv_sb[:, lt, :],
                                start=(lt == 0), stop=(lt == LT - 1))
                        nc.scalar.copy(v_sb[:, blk * (TB // 128) + tt, :, :],
                                       vp[:, 0:KVPC * HEAD_DIM])

                for blk in range(NTB):
                    if blk + 1 < NTB:
                        c0n = (blk + 1) * TB
                        nc.sync.dma_start(
                            hblks[blk + 1][:],
                            hidT[:, c0n:c0n + TB].rearrange("(t p) w -> p t w", p=128))
                    emit_kd(blk)
                    if blk == 1:
                        # first-half latent shards written; gather overlaps
                        # block 1's q sub-pass
                        nc.gpsimd.collective_compute(
                            "AllGather", mybir.AluOpType.bypass, cc_groups,
                            ins=[ckv_shA], outs=[ckv_gA])
                    if blk == NTB - 1:
                        nc.gpsimd.collective_compute(
                            "AllGather", mybir.AluOpType.bypass, cc_groups,
                            ins=[ckv_shB], outs=[ckv_gB])
                    for m in range(QT):
                        emit_q(blk, m)
                        # up-projections interleave into late q sub-passes:
                        # blocks 0-1 during block 2, blocks 2-3 during block 3
                        if blk >= 2 and m in (4, 6):
                            emit_up(2 * (blk - 2) + (m - 4) // 2)

            # ---------------- phase 2: attention ----------------
            with tc.tile_pool(name="wop", bufs=1) as wop, \
                 tc.tile_pool(name="att", bufs=1) as ap_:
                wo_sb = wop.tile([128, QT, HIDDEN], FP, tag="wo")
                attn_sb = ap_.tile([128, QT, NT], FP, tag="attn")

                with tc.tile_pool(name="cst", bufs=1) as cst2, \
                     tc.tile_pool(name="qh", bufs=2) as qhp, \
                     tc.tile_pool(name="pt", bufs=3) as ptp, \
                     tc.tile_pool(name="sac", bufs=2) as sap, \
                     tc.tile_pool(name="sm", bufs=2) as smp, \
                     tc.tile_pool(name="psS", bufs=2, space=MS.PSUM) as psS, \
                     tc.tile_pool(name="psO", bufs=2, space=MS.PSUM) as psO, \
                     tc.tile_pool(name="psU", bufs=2, space=MS.PSUM) as psU:
                    masks_sb = cst2.tile([128, NJ, QB], FP, tag="masks")
                    nc.sync.dma_start(masks_sb[:], masks)
                    ones_sb = cst2.tile([128, 1], FP, tag="ones")
                    nc.sync.dma_start(ones_sb[:], onesd)
                    for m in range(QT):
                        kvh = m // (HPC // KVPC)
                        qh = qhp.tile([128, NT], FP, tag="qh")
                        nc.sync.dma_start(qh[:], qT_s[m])
                        if m == 0:
                            # issue the (large) o_proj weight load behind the
                            # first attention inputs; it has ~500us to land
                            nc.sync.dma_start(
                                wo_sb[:], wo.rearrange("(t p) w -> p t w", p=128))
                        for qb in range(NQB):
                            nkt = (qb + 1) * NJ
                            ops = psO.tile([128, QB], f32, tag="ops")
                            sacc = sap.tile([128, QB], FP, tag="sacc")
                            for p2 in range(nkt // 2):
                                scp = psS.tile([128, 2, QB], f32, tag="scp")
                                for h in range(2):
                                    kt = 2 * p2 + h
                                    nc.tensor.matmul(
                                        scp[:, h, :],
                                        kT[:, kvh, kt * 128:(kt + 1) * 128],
                                        qh[:, qb * QB:(qb + 1) * QB],
                                        start=True, stop=True)
                                pt = ptp.tile([128, 2, QB], FP, tag="pt")
                                nc.scalar.activation(pt[:], scp[:], EXP)
                                jj = 2 * p2 - qb * NJ
                                if jj >= 0:
                                    nc.vector.tensor_mul(pt[:], pt[:],
                                                         masks_sb[:, jj:jj + 2, :])
                                for h in range(2):
                                    kt = 2 * p2 + h
                                    nc.tensor.matmul(
                                        ops[:], v_sb[:, kt, kvh, :], pt[:, h, :],
                                        start=(kt == 0), stop=(kt == nkt - 1))
                                if p2 == 0:
                                    nc.vector.tensor_copy(sacc[:], pt[:, 0, :])
                                else:
                                    nc.vector.tensor_add(sacc[:], sacc[:], pt[:, 0, :])
                                nc.vector.tensor_add(sacc[:], sacc[:], pt[:, 1, :])
                            sps = psU.tile([1, QB], f32, tag="sps")
                            nc.tensor.matmul(sps[:], ones_sb[:], sacc[:],
                                             start=True, stop=True)
                            srow = smp.tile([1, QB], f32, tag="srow")
                            nc.scalar.copy(srow[:], sps[:])
                            sbc = smp.tile([128, QB], f32, tag="sbc")
                            nc.gpsimd.partition_broadcast(sbc[:], srow[:])
                            rbc = smp.tile([128, QB], f32, tag="rbc")
                            nc.vector.reciprocal_approx_fast(rbc[:], sbc[:])
                            nc.vector.tensor_mul(
                                attn_sb[:, m, qb * QB:(qb + 1) * QB],
                                ops[:], rbc[:])

                # ---------------- phase 3: partial o_proj ----------------
                with tc.tile_pool(name="st4", bufs=4) as st4, \
                     tc.tile_pool(name="ps4", bufs=8, space=MS.PSUM) as ps4:
                    for n in range(HT):
                        for j in range(NQB):
                            ps = ps4.tile([128, QB], f32, tag="ps")
                            for h2 in range(QT):
                                nc.tensor.matmul(
                                    ps[:],
                                    wo_sb[:, h2, n * 128:(n + 1) * 128],
                                    attn_sb[:, h2, j * QB:(j + 1) * QB],
                                    start=(h2 == 0), stop=(h2 == QT - 1))
                            osb = st4.tile([128, QB], FP, tag="osb")
                            nc.vector.tensor_copy(osb[:], ps[:])
                            nc.sync.dma_start(
                                outp[n * 128:(n + 1) * 128, j * QB:(j + 1) * QB],
                                osb[:])

    nc.compile()
    return nc


def make_in_maps(hidden_states, Wq, Wkr, Wdk, Wupk, Wupv, Wo, Sv=S, QB=512):
    """Host-side sharding + layout prep. Returns per-core input dicts."""
    NJ = QB // 128
    scale = 1.0 / np.sqrt(np.float32(HEAD_DIM))
    hidden_states = np.asarray(hidden_states, np.float32)
    Wq, Wkr, Wdk = np.asarray(Wq, np.float32), np.asarray(Wkr, np.float32), np.asarray(Wdk, np.float32)
    Wupk, Wupv, Wo = np.asarray(Wupk, np.float32), np.asarray(Wupv, np.float32), np.asarray(Wo, np.float32)

    cos_t, sin_t = _rope_tables(Sv)                    # [128, S]
    qcos = np.ascontiguousarray(cos_t * scale).astype(F16)
    qsin = np.ascontiguousarray(
        np.concatenate([-sin_t[0:64], sin_t[64:128]], axis=0) * scale).astype(F16)
    # per kv head 64 rope rows = [dims 0:32, dims 64:96], tiled x KVPC
    kc1 = np.concatenate([cos_t[0:32], cos_t[64:96]], axis=0)
    ks1 = np.concatenate([-sin_t[0:32], sin_t[64:96]], axis=0)
    kcos = np.ascontiguousarray(np.tile(kc1, (KVPC, 1))).astype(F16)
    ksin = np.ascontiguousarray(np.tile(ks1, (KVPC, 1))).astype(F16)

    k_idx = np.arange(128)[:, None]
    q_idx = np.arange(QB)[None, :]
    masks = np.stack(
        [(q_idx >= j * 128 + k_idx).astype(np.float32) for j in range(NJ)],
        axis=1)                                        # [128, NJ, QB]
    masks = np.ascontiguousarray(masks).astype(F16)

    in_maps = []
    for c in range(NCORES):
        b, tp = c // TPG, c % TPG
        hidT = np.ascontiguousarray(hidden_states[b].T).astype(F16)
        wq_t = np.ascontiguousarray(Wq[QR * tp:QR * (tp + 1)].T).astype(F16)
        wkd_t = np.ascontiguousarray(
            np.concatenate([Wdk[CDS * tp:CDS * (tp + 1)],
                            Wkr[KRR * tp:KRR * (tp + 1)]], axis=0).T).astype(F16)
        wupk_t = np.ascontiguousarray(Wupk[KRR * tp:KRR * (tp + 1)].T).astype(F16)
        wupv_t = np.ascontiguousarray(
            Wupv[KVPC * HEAD_DIM * tp:KVPC * HEAD_DIM * (tp + 1)].T).astype(F16)
        wo_t = np.ascontiguousarray(Wo[:, QR * tp:QR * (tp + 1)].T).astype(F16)
        in_maps.append({
            "hidT": hidT, "wq_t": wq_t, "wkd_t": wkd_t,
            "wupk_t": wupk_t, "wupv_t": wupv_t, "wo_t": wo_t,
            "qcos": qcos, "qsin": qsin, "kcos": kcos, "ksin": ksin,
            "masks": masks, "ones": np.ones((128, 1), F16),
        })
    return in_maps


def combine_outputs(res):
    outs = []
    for b in range(B):
        acc = res.results[b * TPG]["out_part"].astype(np.float32)
        for tp in range(1, TPG):
            acc = acc + res.results[b * TPG + tp]["out_part"].astype(np.float32)
        outs.append(acc.T)                             # [S, HIDDEN]
    return np.stack(outs).astype(np.float32)           # [B, S, HIDDEN]


_NC_CACHE = {}


def _get_program(key=(S, 512, 512)):
    if key not in _NC_CACHE:
        _NC_CACHE[key] = build_program(*key)
    return _NC_CACHE[key]


def kernel(hidden_states, Wq, Wkr, Wdk, Wupk, Wupv, Wo):
    from concourse.bass_utils import run_bass_kernel_spmd

    in_maps = make_in_maps(np.asarray(hidden_states), Wq, Wkr, Wdk, Wupk, Wupv, Wo)
    nc = _get_program()
    res = run_bass_kernel_spmd(nc, in_maps, list(range(NCORES)))
    return combine_outputs(res)


# revision 27
# speedup vs baseline: 1.5449x; 1.0149x over previous
"""MLA (CustomLlamaMLAForInfer) Trainium2 Bass kernel, v4.

Sharding: batch x tensor-parallel. Core c owns batch c//4 and TP shard
tp=c%4: 8 q-heads, 2 kv-heads, 1024 Wo columns, and a 128-row shard of
the shared latent projection Wdk. Latent shards are AllGathered within
each batch group of 4 on device. o_proj partials ([HIDDEN, 2048] fp16,
transposed layout) are summed per batch group on the host.

All matmul operands are fp16 (fp32 PSUM accumulation); rope math runs
in fp32 on the DVE at PSUM-evict time. Softmax denominators accumulate
on the DVE in fp16 (scores are bounded, exp stays in fp16 range) and
are reduced across partitions with one ones-matmul per (head, q-block).

Device phases (single SPMD program; per-core weights differ):
  1. per 512-token block: latent-shard + k-rope projections (2 PSUM
     tiles), q projection (8 PSUM tiles, head-outer so evictions
     pipeline), rope at evict. qT spills to DRAM fp16.
  1g. AllGather latent shards -> full 512-dim latent (DRAM, fp16).
  1u. per block: k_nope/v up-projections from the gathered latent.
  2. causal attention per (q-head, q-block): paired score tiles, one
     exp per pair, diag masks, PV with v-stationary matmuls,
     fp16 DVE denominator accumulation, approx reciprocal.
  3. partial o_proj in [hid, tok] layout, fp16 output.
"""

import numpy as np

HIDDEN = 4096
N_HEADS = 32
KV_HEADS = 8
HEAD_DIM = 128
LOW_RANK = 64
TOP_K_ROPE = 32
ROPE_THETA = 10000.0
B, S = 2, 2048
NCORES = 8
TPG = 4                      # TP group size (cores per batch)
HPC = N_HEADS // TPG         # q heads per core = 8
KVPC = KV_HEADS // TPG       # kv heads per core = 2
QR = HPC * HEAD_DIM          # q rows per core = 1024
CD = LOW_RANK * KV_HEADS     # latent dim = 512
CDS = CD // TPG              # latent shard rows per core = 128
KRR = 64 * KVPC              # rope rows per core = 128
F16 = np.float16


def _rope_tables(seq_len):
    inv = 1.0 / (ROPE_THETA ** (np.arange(0, HEAD_DIM, 2, dtype=np.float32) / HEAD_DIM))
    pos = np.arange(seq_len, dtype=np.float32)
    fr = np.outer(pos, inv)
    emb = np.concatenate([fr, fr], axis=-1)          # [S, 128]
    return (np.cos(emb).T.astype(np.float32),        # [128, S]
            np.sin(emb).T.astype(np.float32))


def build_program(Sv=S, TB=512, QB=512):
    from concourse import bacc, tile, mybir
    import concourse.bass as bass

    f32 = mybir.dt.float32
    FP = mybir.dt.float16
    MS = bass.MemorySpace
    EXP = mybir.ActivationFunctionType.Exp

    NT = Sv                      # tokens per core (one batch)
    HT = HIDDEN // 128           # hidden tiles = 32
    NTB = NT // TB               # proj token blocks = 4
    NQB = NT // QB               # attention q blocks = 4
    NJ = QB // 128               # diagonal mask variants = 4
    QT = HPC                     # q-head tiles = 8
    LT = CD // 128               # latent tiles = 4
    NKT = NT // 128              # k tiles = 16

    nc = bacc.Bacc("TRN2", target_bir_lowering=False, debug=False,
                   num_devices=NCORES)

    def din(name, shape, dt=FP):
        return nc.dram_tensor(name, shape, dt, kind="ExternalInput").ap()

    hidT = din("hidT", [HIDDEN, NT])
    wq = din("wq_t", [HIDDEN, QR])
    wkd = din("wkd_t", [HIDDEN, CDS + KRR])
    wupk = din("wupk_t", [CD, KRR])
    wupv = din("wupv_t", [CD, KVPC * HEAD_DIM])
    wo = din("wo_t", [QR, HIDDEN])
    qcos = din("qcos", [128, NT])
    qsin = din("qsin", [128, NT])
    kcos = din("kcos", [128, NT])
    ksin = din("ksin", [128, NT])
    masks = din("masks", [128, NJ, QB])
    onesd = din("ones", [128, 1])
    outp = nc.dram_tensor("out_part", [HIDDEN, NT], FP, kind="ExternalOutput").ap()
    qT_s = nc.dram_tensor("qT_s", [QT, 128, NT], FP).ap()
    NH = NT // 2
    ckv_shA = nc.dram_tensor("ckv_shA", [128, NH], FP).ap()
    ckv_shB = nc.dram_tensor("ckv_shB", [128, TB], FP).ap()
    ckv_shC = nc.dram_tensor("ckv_shC", [128, TB], FP).ap()
    ckv_gA = nc.dram_tensor("ckv_gA", [TPG, 128, NH], FP).ap()
    ckv_gB = nc.dram_tensor("ckv_gB", [TPG, 128, TB], FP).ap()
    ckv_gC = nc.dram_tensor("ckv_gC", [TPG, 128, TB], FP).ap()
    cc_groups = [[g * TPG + i for i in range(TPG)] for g in range(NCORES // TPG)]

    with tile.TileContext(nc) as tc:
        with tc.tile_pool(name="persist", bufs=1) as pers:
            kT = pers.tile([128, KVPC, NT], FP, tag="kT")
            v_sb = pers.tile([128, NKT, KVPC, HEAD_DIM], FP, tag="v")

            # ---------------- phase 1: hid projections ----------------
            with tc.tile_pool(name="tabs", bufs=1) as tbp, \
                 tc.tile_pool(name="w1", bufs=1) as w1, \
                 tc.tile_pool(name="hb", bufs=2) as hbp, \
                 tc.tile_pool(name="cbp", bufs=2) as cbp, \
                 tc.tile_pool(name="st1", bufs=2) as st1, \
                 tc.tile_pool(name="ps1", bufs=8, space=MS.PSUM) as ps1:
                # DMA issue order matters for the cold start: the first kd
                # sub-pass needs only wkd + the first hid block.
                wq_sb = w1.tile([128, HT, QR], FP, tag="wq")
                wkd_sb = w1.tile([128, HT, CDS + KRR], FP, tag="wkd")
                wupk_sb = w1.tile([128, LT, KRR], FP, tag="upk")
                wupv_sb = w1.tile([128, LT, KVPC * HEAD_DIM], FP, tag="upv")
                nc.sync.dma_start(
                    wkd_sb[:, 0:HT // 2, :],
                    wkd[0:HIDDEN // 2].rearrange("(t p) w -> p t w", p=128))
                hblks = [hbp.tile([128, HT, TB], FP, tag="hid", name=f"hb{_b}")
                         for _b in range(NTB)]
                nc.sync.dma_start(
                    hblks[0][:, 0:HT // 2, :],
                    hidT[0:HIDDEN // 2, 0:TB].rearrange("(t p) w -> p t w", p=128))
                nc.sync.dma_start(
                    wkd_sb[:, HT // 2:, :],
                    wkd[HIDDEN // 2:].rearrange("(t p) w -> p t w", p=128))
                nc.sync.dma_start(
                    hblks[0][:, HT // 2:, :],
                    hidT[HIDDEN // 2:, 0:TB].rearrange("(t p) w -> p t w", p=128))
                for _qtr in range(4):
                    w0, w1c = _qtr * (QR // 4), (_qtr + 1) * (QR // 4)
                    nc.sync.dma_start(
                        wq_sb[:, :, w0:w1c],
                        wq[:, w0:w1c].rearrange("(t p) w -> p t w", p=128))
                qcos_sb = tbp.tile([128, NT], FP, tag="qc")
                qsin_sb = tbp.tile([128, NT], FP, tag="qs")
                kcos_sb = tbp.tile([128, NT], FP, tag="kc")
                ksin_sb = tbp.tile([128, NT], FP, tag="ks")
                nc.sync.dma_start(kcos_sb[:], kcos)
                nc.sync.dma_start(ksin_sb[:], ksin)
                nc.sync.dma_start(qcos_sb[:], qcos)
                nc.sync.dma_start(qsin_sb[:], qsin)
                nc.sync.dma_start(wupk_sb[:], wupk.rearrange("(t p) w -> p t w", p=128))
                nc.sync.dma_start(wupv_sb[:], wupv.rearrange("(t p) w -> p t w", p=128))

                def emit_kd(blk):
                    c0, c1 = blk * TB, (blk + 1) * TB
                    hblk = hblks[blk]
                    # kd sub-pass: latent shard (1 tile) + k-rope (1 tile)
                    csp = ps1.tile([128, TB], f32, tag="ps", name="csp")
                    krp = ps1.tile([128, TB], f32, tag="ps", name="krp")
                    for t in range(HT):
                        nc.tensor.matmul(
                            csp[:], wkd_sb[:, t, 0:CDS], hblk[:, t, :],
                            start=(t == 0), stop=(t == HT - 1))
                        nc.tensor.matmul(
                            krp[:], wkd_sb[:, t, CDS:CDS + KRR], hblk[:, t, :],
                            start=(t == 0), stop=(t == HT - 1))
                    cst = st1.tile([128, TB], FP, tag="cst")
                    nc.scalar.copy(cst[:], csp[:])
                    sh, s0 = [(ckv_shA, c0), (ckv_shA, c0),
                              (ckv_shB, 0), (ckv_shC, 0)][blk]
                    nc.sync.dma_start(sh[:, s0:s0 + TB], cst[:])
                    # k-rope rows: per kv head 64 rows = [dims 0:32, dims 64:96]
                    rawk = st1.tile([128, TB], f32, tag="rawk")
                    nc.scalar.copy(rawk[:], krp[:])
                    rotk = st1.tile([128, TB], f32, tag="rotk")
                    nc.sync.dma_start(rotk[0:32, :], rawk[32:64, :])
                    nc.sync.dma_start(rotk[32:64, :], rawk[0:32, :])
                    nc.sync.dma_start(rotk[64:96, :], rawk[96:128, :])
                    nc.sync.dma_start(rotk[96:128, :], rawk[64:96, :])
                    nc.vector.tensor_mul(rawk[:], rawk[:], kcos_sb[:, c0:c1])
                    nc.vector.tensor_mul(rotk[:], rotk[:], ksin_sb[:, c0:c1])
                    for kv in range(KVPC):
                        r0 = kv * 64
                        nc.vector.tensor_add(kT[0:32, kv, c0:c1],
                                             rawk[r0:r0 + 32, :], rotk[r0:r0 + 32, :])
                        nc.vector.tensor_add(kT[64:96, kv, c0:c1],
                                             rawk[r0 + 32:r0 + 64, :], rotk[r0 + 32:r0 + 64, :])

                def emit_q(blk, m):
                    c0, c1 = blk * TB, (blk + 1) * TB
                    hblk = hblks[blk]
                    qp = ps1.tile([128, TB], f32, tag="ps", name=f"qp{m}")
                    for t in range(HT):
                        nc.tensor.matmul(
                            qp[:], wq_sb[:, t, m * 128:(m + 1) * 128],
                            hblk[:, t, :], start=(t == 0), stop=(t == HT - 1))
                    raw = st1.tile([128, TB], f32, tag="qraw")
                    nc.scalar.copy(raw[:], qp[:])
                    rot = st1.tile([128, TB], f32, tag="qrot")
                    nc.sync.dma_start(rot[0:64, :], raw[64:128, :])
                    nc.sync.dma_start(rot[64:128, :], raw[0:64, :])
                    nc.vector.tensor_mul(raw[:], raw[:], qcos_sb[:, c0:c1])
                    nc.vector.tensor_mul(rot[:], rot[:], qsin_sb[:, c0:c1])
                    qsb = st1.tile([128, TB], FP, tag="qsb")
                    nc.vector.tensor_add(qsb[:], raw[:], rot[:])
                    nc.sync.dma_start(qT_s[m, :, c0:c1], qsb[:])

                def emit_up(blk):
                    c0, c1 = blk * TB, (blk + 1) * TB
                    g, g0 = [(ckv_gA, 0), (ckv_gA, TB),
                             (ckv_gB, 0), (ckv_gC, 0)][blk]
                    cb = cbp.tile([128, LT, TB], FP, tag="cb")
                    nc.sync.dma_start(
                        cb[:], g[:, :, g0:g0 + TB].rearrange("g p w -> p g w"))
                    kcp = ps1.tile([128, TB], f32, tag="ps", name="kcp")
                    for lt in range(LT):
                        nc.tensor.matmul(kcp[:], wupk_sb[:, lt, :], cb[:, lt, :],
                                         start=(lt == 0), stop=(lt == LT - 1))
                    # rows: [kv0 d32:64, kv0 d96:128, kv1 d32:64, kv1 d96:128]
                    for kv in range(KVPC):
                        r0 = kv * 64
                        nc.scalar.copy(kT[32:64, kv, c0:c1], kcp[r0:r0 + 32, :])
                        nc.scalar.copy(kT[96:128, kv, c0:c1], kcp[r0 + 32:r0 + 64, :])
                    for tt in range(TB // 128):
                        vp = ps1.tile([128, TB], f32, tag="ps", name=f"vp{tt}")
                        for lt in range(LT):
                            nc.tensor.matmul(
                                vp[:, 0:KVPC * HEAD_DIM],
                                cb[:, lt, tt * 128:(tt + 1) * 128],
                                wupv_sb[:, lt, :],
                                start=(lt == 0), stop=(lt == LT - 1))
                        nc.scalar.copy(v_sb[:, blk * (TB // 128) + tt, :, :],
                                       vp[:, 0:KVPC * HEAD_DIM])

                for blk in range(NTB):
                    if blk + 1 < NTB:
                        c0n = (blk + 1) * TB
                        nc.sync.dma_start(
                            hblks[blk + 1][:],
                            hidT[:, c0n:c0n + TB].rearrange("(t p) w -> p t w", p=128))
                    emit_kd(blk)
                    # gathers issue as soon as their shard blocks are written,
                    # overlapping the q sub-passes
                    if blk == 1:
                        nc.gpsimd.collective_compute(
                            "AllGather", mybir.AluOpType.bypass, cc_groups,
                            ins=[ckv_shA], outs=[ckv_gA])
                    elif blk == 2:
                        nc.gpsimd.collective_compute(
                            "AllGather", mybir.AluOpType.bypass, cc_groups,
                            ins=[ckv_shB], outs=[ckv_gB])
                    elif blk == 3:
                        nc.gpsimd.collective_compute(
                            "AllGather", mybir.AluOpType.bypass, cc_groups,
                            ins=[ckv_shC], outs=[ckv_gC])
                    for m in range(QT):
                        emit_q(blk, m)
                        # up-projections interleave into late q sub-passes:
                        # blocks 0-1 during block 2, blocks 2-3 during block 3
                        if blk >= 2 and m in (4, 6):
                            emit_up(2 * (blk - 2) + (m - 4) // 2)

            # ---------------- phase 2: attention ----------------
            with tc.tile_pool(name="wop", bufs=1) as wop, \
                 tc.tile_pool(name="att", bufs=1) as ap_:
                wo_sb = wop.tile([128, QT, HIDDEN], FP, tag="wo")
                attn_sb = ap_.tile([128, QT, NT], FP, tag="attn")

                with tc.tile_pool(name="cst", bufs=1) as cst2, \
                     tc.tile_pool(name="qh", bufs=2) as qhp, \
                     tc.tile_pool(name="pt", bufs=3) as ptp, \
                     tc.tile_pool(name="sac", bufs=2) as sap, \
                     tc.tile_pool(name="sm", bufs=2) as smp, \
                     tc.tile_pool(name="psS", bufs=2, space=MS.PSUM) as psS, \
                     tc.tile_pool(name="psO", bufs=3, space=MS.PSUM) as psO, \
                     tc.tile_pool(name="psU", bufs=1, space=MS.PSUM) as psU:
                    masks_sb = cst2.tile([128, NJ, QB], FP, tag="masks")
                    nc.sync.dma_start(masks_sb[:], masks)
                    ones_sb = cst2.tile([128, 1], FP, tag="ones")
                    nc.sync.dma_start(ones_sb[:], onesd)
                    qhs = [qhp.tile([128, NT], FP, tag="qh", name=f"qh{_m}")
                           for _m in range(QT)]
                    nc.sync.dma_start(qhs[0][:], qT_s[0])
                    for m in range(QT):
                        kvh = m // (HPC // KVPC)
                        qh = qhs[m]
                        if m == 0:
                            # issue the (large) o_proj weight load behind the
                            # first attention inputs; it has ~500us to land
                            nc.sync.dma_start(
                                wo_sb[:], wo.rearrange("(t p) w -> p t w", p=128))
                        # 4 denominator rows share one PSUM bank at partition
                        # offsets {0,32,64,96}
                        ubank = psU.tile([128, QB], f32, tag="sps")
                        for qb in range(NQB):
                            nkt = (qb + 1) * NJ
                            ops = psO.tile([128, QB], f32, tag="ops")
                            sacc = sap.tile([128, QB], FP, tag="sacc")
                            if qb == 0 and m + 1 < QT:
                                nc.sync.dma_start(qhs[m + 1][:], qT_s[m + 1])
                            for p2 in range(nkt // 2):
                                scp = psS.tile([128, 2, QB], f32, tag="scp")
                                for h in range(2):
                                    kt = 2 * p2 + h
                                    nc.tensor.matmul(
                                        scp[:, h, :],
                                        kT[:, kvh, kt * 128:(kt + 1) * 128],
                                        qh[:, qb * QB:(qb + 1) * QB],
                                        start=True, stop=True)
                                pt = ptp.tile([128, 2, QB], FP, tag="pt")
                                nc.scalar.activation(pt[:], scp[:], EXP)
                                jj = 2 * p2 - qb * NJ
                                if jj >= 0:
                                    nc.vector.tensor_mul(pt[:], pt[:],
                                                         masks_sb[:, jj:jj + 2, :])
                                for h in range(2):
                                    kt = 2 * p2 + h
                                    nc.tensor.matmul(
                                        ops[:], v_sb[:, kt, kvh, :], pt[:, h, :],
                                        start=(kt == 0), stop=(kt == nkt - 1))
                                if p2 == 0:
                                    nc.vector.tensor_copy(sacc[:], pt[:, 0, :])
                                else:
                                    nc.vector.tensor_add(sacc[:], sacc[:], pt[:, 0, :])
                                nc.vector.tensor_add(sacc[:], sacc[:], pt[:, 1, :])
                            u0 = (32 * qb) % 96   # offsets 0/32/64; qb=3 reuses 0
                            sps = ubank[u0:u0 + 1, :]
                            nc.tensor.matmul(sps, ones_sb[:], sacc[:],
                                             start=True, stop=True)
                            srow = smp.tile([1, QB], f32, tag="srow")
                            nc.scalar.copy(srow[:], sps)
                            sbc = smp.tile([128, QB], f32, tag="sbc")
                            nc.gpsimd.partition_broadcast(sbc[:], srow[:])
                            rbc = smp.tile([128, QB], f32, tag="rbc")
                            nc.vector.reciprocal_approx_fast(rbc[:], sbc[:])
                            nc.vector.tensor_mul(
                                attn_sb[:, m, qb * QB:(qb + 1) * QB],
                                ops[:], rbc[:])

                # ---------------- phase 3: partial o_proj ----------------
                with tc.tile_pool(name="st4", bufs=4) as st4, \
                     tc.tile_pool(name="ps4", bufs=8, space=MS.PSUM) as ps4:
                    for n in range(HT):
                        for j in range(NQB):
                            ps = ps4.tile([128, QB], f32, tag="ps")
                            for h2 in range(QT):
                                nc.tensor.matmul(
                                    ps[:],
                                    wo_sb[:, h2, n * 128:(n + 1) * 128],
                                    attn_sb[:, h2, j * QB:(j + 1) * QB],
                                    start=(h2 == 0), stop=(h2 == QT - 1))
                            osb = st4.tile([128, QB], FP, tag="osb")
                            nc.vector.tensor_copy(osb[:], ps[:])
                            nc.sync.dma_start(
                                outp[n * 128:(n + 1) * 128, j * QB:(j + 1) * QB],
                                osb[:])

    nc.compile()
    return nc


def make_in_maps(hidden_states, Wq, Wkr, Wdk, Wupk, Wupv, Wo, Sv=S, QB=512):
    """Host-side sharding + layout prep. Returns per-core input dicts."""
    NJ = QB // 128
    scale = 1.0 / np.sqrt(np.float32(HEAD_DIM))
    hidden_states = np.asarray(hidden_states, np.float32)
    Wq, Wkr, Wdk = np.asarray(Wq, np.float32), np.asarray(Wkr, np.float32), np.asarray(Wdk, np.float32)
    Wupk, Wupv, Wo = np.asarray(Wupk, np.float32), np.asarray(Wupv, np.float32), np.asarray(Wo, np.float32)

    cos_t, sin_t = _rope_tables(Sv)                    # [128, S]
    qcos = np.ascontiguousarray(cos_t * scale).astype(F16)
    qsin = np.ascontiguousarray(
        np.concatenate([-sin_t[0:64], sin_t[64:128]], axis=0) * scale).astype(F16)
    # per kv head 64 rope rows = [dims 0:32, dims 64:96], tiled x KVPC
    kc1 = np.concatenate([cos_t[0:32], cos_t[64:96]], axis=0)
    ks1 = np.concatenate([-sin_t[0:32], sin_t[64:96]], axis=0)
    kcos = np.ascontiguousarray(np.tile(kc1, (KVPC, 1))).astype(F16)
    ksin = np.ascontiguousarray(np.tile(ks1, (KVPC, 1))).astype(F16)

    k_idx = np.arange(128)[:, None]
    q_idx = np.arange(QB)[None, :]
    masks = np.stack(
        [(q_idx >= j * 128 + k_idx).astype(np.float32) for j in range(NJ)],
        axis=1)                                        # [128, NJ, QB]
    masks = np.ascontiguousarray(masks).astype(F16)

    in_maps = []
    for c in range(NCORES):
        b, tp = c // TPG, c % TPG
        hidT = np.ascontiguousarray(hidden_states[b].T).astype(F16)
        wq_t = np.ascontiguousarray(Wq[QR * tp:QR * (tp + 1)].T).astype(F16)
        wkd_t = np.ascontiguousarray(
            np.concatenate([Wdk[CDS * tp:CDS * (tp + 1)],
                            Wkr[KRR * tp:KRR * (tp + 1)]], axis=0).T).astype(F16)
        wupk_t = np.ascontiguousarray(Wupk[KRR * tp:KRR * (tp + 1)].T).astype(F16)
        wupv_t = np.ascontiguousarray(
            Wupv[KVPC * HEAD_DIM * tp:KVPC * HEAD_DIM * (tp + 1)].T).astype(F16)
        wo_t = np.ascontiguousarray(Wo[:, QR * tp:QR * (tp + 1)].T).astype(F16)
        in_maps.append({
            "hidT": hidT, "wq_t": wq_t, "wkd_t": wkd_t,
            "wupk_t": wupk_t, "wupv_t": wupv_t, "wo_t": wo_t,
            "qcos": qcos, "qsin": qsin, "kcos": kcos, "ksin": ksin,
            "masks": masks, "ones": np.ones((128, 1), F16),
        })
    return in_maps


def combine_outputs(res):
    outs = []
    for b in range(B):
        acc = res.results[b * TPG]["out_part"].astype(np.float32)
        for tp in range(1, TPG):
            acc = acc + res.results[b * TPG + tp]["out_part"].astype(np.float32)
        outs.append(acc.T)                             # [S, HIDDEN]
    return np.stack(outs).astype(np.float32)           # [B, S, HIDDEN]


_NC_CACHE = {}


def _get_program(key=(S, 512, 512)):
    if key not in _NC_CACHE:
        _NC_CACHE[key] = build_program(*key)
    return _NC_CACHE[key]


def kernel(hidden_states, Wq, Wkr, Wdk, Wupk, Wupv, Wo):
    from concourse.bass_utils import run_bass_kernel_spmd

    in_maps = make_in_maps(np.asarray(hidden_states), Wq, Wkr, Wdk, Wupk, Wupv, Wo)
    nc = _get_program()
    res = run_bass_kernel_spmd(nc, in_maps, list(range(NCORES)))
    return combine_outputs(res)


# revision 28
# speedup vs baseline: 1.5767x; 1.0206x over previous
"""MLA (CustomLlamaMLAForInfer) Trainium2 Bass kernel, v4.

Sharding: batch x tensor-parallel. Core c owns batch c//4 and TP shard
tp=c%4: 8 q-heads, 2 kv-heads, 1024 Wo columns, and a 128-row shard of
the shared latent projection Wdk. Latent shards are AllGathered within
each batch group of 4 on device. o_proj partials ([HIDDEN, 2048] fp16,
transposed layout) are summed per batch group on the host.

All matmul operands are fp16 (fp32 PSUM accumulation); rope math runs
in fp32 on the DVE at PSUM-evict time. Softmax denominators accumulate
on the DVE in fp16 (scores are bounded, exp stays in fp16 range) and
are reduced across partitions with one ones-matmul per (head, q-block).

Device phases (single SPMD program; per-core weights differ):
  1. per 512-token block: latent-shard + k-rope projections (2 PSUM
     tiles), q projection (8 PSUM tiles, head-outer so evictions
     pipeline), rope at evict. qT spills to DRAM fp16.
  1g. AllGather latent shards -> full 512-dim latent (DRAM, fp16).
  1u. per block: k_nope/v up-projections from the gathered latent.
  2. causal attention per (q-head, q-block): paired score tiles, one
     exp per pair, diag masks, PV with v-stationary matmuls,
     fp16 DVE denominator accumulation, approx reciprocal.
  3. partial o_proj in [hid, tok] layout, fp16 output.
"""

import numpy as np

HIDDEN = 4096
N_HEADS = 32
KV_HEADS = 8
HEAD_DIM = 128
LOW_RANK = 64
TOP_K_ROPE = 32
ROPE_THETA = 10000.0
B, S = 2, 2048
NCORES = 8
TPG = 4                      # TP group size (cores per batch)
HPC = N_HEADS // TPG         # q heads per core = 8
KVPC = KV_HEADS // TPG       # kv heads per core = 2
QR = HPC * HEAD_DIM          # q rows per core = 1024
CD = LOW_RANK * KV_HEADS     # latent dim = 512
CDS = CD // TPG              # latent shard rows per core = 128
KRR = 64 * KVPC              # rope rows per core = 128
F16 = np.float16


def _rope_tables(seq_len):
    inv = 1.0 / (ROPE_THETA ** (np.arange(0, HEAD_DIM, 2, dtype=np.float32) / HEAD_DIM))
    pos = np.arange(seq_len, dtype=np.float32)
    fr = np.outer(pos, inv)
    emb = np.concatenate([fr, fr], axis=-1)          # [S, 128]
    return (np.cos(emb).T.astype(np.float32),        # [128, S]
            np.sin(emb).T.astype(np.float32))


def build_program(Sv=S, TB=512, QB=512):
    from concourse import bacc, tile, mybir
    import concourse.bass as bass

    f32 = mybir.dt.float32
    FP = mybir.dt.float16
    MS = bass.MemorySpace
    EXP = mybir.ActivationFunctionType.Exp

    NT = Sv                      # tokens per core (one batch)
    HT = HIDDEN // 128           # hidden tiles = 32
    NTB = NT // TB               # proj token blocks = 4
    NQB = NT // QB               # attention q blocks = 4
    NJ = QB // 128               # diagonal mask variants = 4
    QT = HPC                     # q-head tiles = 8
    LT = CD // 128               # latent tiles = 4
    NKT = NT // 128              # k tiles = 16

    nc = bacc.Bacc("TRN2", target_bir_lowering=False, debug=False,
                   num_devices=NCORES)

    def din(name, shape, dt=FP):
        return nc.dram_tensor(name, shape, dt, kind="ExternalInput").ap()

    hidT = din("hidT", [HIDDEN, NT])
    wq = din("wq_t", [HIDDEN, QR])
    wkd = din("wkd_t", [HIDDEN, CDS + KRR])
    wupk = din("wupk_t", [CD, KRR])
    wupv = din("wupv_t", [CD, KVPC * HEAD_DIM])
    wo = din("wo_t", [QR, HIDDEN])
    qcos = din("qcos", [128, NT])
    qsin = din("qsin", [128, NT])
    kcos = din("kcos", [128, NT])
    ksin = din("ksin", [128, NT])
    masks = din("masks", [128, NJ, QB])
    onesd = din("ones", [128, 1])
    outp = nc.dram_tensor("out_part", [HIDDEN, NT], FP, kind="ExternalOutput").ap()
    qT_s = nc.dram_tensor("qT_s", [QT, 128, NT], FP).ap()
    NH = NT // 2
    ckv_shA = nc.dram_tensor("ckv_shA", [128, NH], FP).ap()
    ckv_shB = nc.dram_tensor("ckv_shB", [128, TB], FP).ap()
    ckv_shC = nc.dram_tensor("ckv_shC", [128, TB], FP).ap()
    ckv_gA = nc.dram_tensor("ckv_gA", [TPG, 128, NH], FP).ap()
    ckv_gB = nc.dram_tensor("ckv_gB", [TPG, 128, TB], FP).ap()
    ckv_gC = nc.dram_tensor("ckv_gC", [TPG, 128, TB], FP).ap()
    cc_groups = [[g * TPG + i for i in range(TPG)] for g in range(NCORES // TPG)]

    with tile.TileContext(nc) as tc:
        with tc.tile_pool(name="persist", bufs=1) as pers:
            kT = pers.tile([128, KVPC, NT], FP, tag="kT")
            v_sb = pers.tile([128, NKT, KVPC, HEAD_DIM], FP, tag="v")

            # ---------------- phase 1: hid projections ----------------
            with tc.tile_pool(name="tabs", bufs=1) as tbp, \
                 tc.tile_pool(name="w1", bufs=1) as w1, \
                 tc.tile_pool(name="hb", bufs=2) as hbp, \
                 tc.tile_pool(name="cbp", bufs=2) as cbp, \
                 tc.tile_pool(name="st1", bufs=2) as st1, \
                 tc.tile_pool(name="ps1", bufs=8, space=MS.PSUM) as ps1:
                # DMA issue order matters for the cold start: the first kd
                # sub-pass needs only wkd + the first hid block.
                wq_sb = w1.tile([128, HT, QR], FP, tag="wq")
                wkd_sb = w1.tile([128, HT, CDS + KRR], FP, tag="wkd")
                wupk_sb = w1.tile([128, LT, KRR], FP, tag="upk")
                wupv_sb = w1.tile([128, LT, KVPC * HEAD_DIM], FP, tag="upv")
                nc.sync.dma_start(
                    wkd_sb[:, 0:HT // 2, :],
                    wkd[0:HIDDEN // 2].rearrange("(t p) w -> p t w", p=128))
                hblks = [hbp.tile([128, HT, TB], FP, tag="hid", name=f"hb{_b}")
                         for _b in range(NTB)]
                nc.sync.dma_start(
                    hblks[0][:, 0:HT // 2, :],
                    hidT[0:HIDDEN // 2, 0:TB].rearrange("(t p) w -> p t w", p=128))
                nc.sync.dma_start(
                    wkd_sb[:, HT // 2:, :],
                    wkd[HIDDEN // 2:].rearrange("(t p) w -> p t w", p=128))
                nc.sync.dma_start(
                    hblks[0][:, HT // 2:, :],
                    hidT[HIDDEN // 2:, 0:TB].rearrange("(t p) w -> p t w", p=128))
                for _qtr in range(4):
                    w0, w1c = _qtr * (QR // 4), (_qtr + 1) * (QR // 4)
                    nc.sync.dma_start(
                        wq_sb[:, :, w0:w1c],
                        wq[:, w0:w1c].rearrange("(t p) w -> p t w", p=128))
                qcos_sb = tbp.tile([128, NT], FP, tag="qc")
                qsin_sb = tbp.tile([128, NT], FP, tag="qs")
                kcos_sb = tbp.tile([128, NT], FP, tag="kc")
                ksin_sb = tbp.tile([128, NT], FP, tag="ks")
                nc.sync.dma_start(kcos_sb[:], kcos)
                nc.sync.dma_start(ksin_sb[:], ksin)
                nc.sync.dma_start(qcos_sb[:], qcos)
                nc.sync.dma_start(qsin_sb[:], qsin)
                nc.sync.dma_start(wupk_sb[:], wupk.rearrange("(t p) w -> p t w", p=128))
                nc.sync.dma_start(wupv_sb[:], wupv.rearrange("(t p) w -> p t w", p=128))

                def emit_kd(blk):
                    c0, c1 = blk * TB, (blk + 1) * TB
                    hblk = hblks[blk]
                    # kd sub-pass: latent shard (1 tile) + k-rope (1 tile)
                    csp = ps1.tile([128, TB], f32, tag="ps", name="csp")
                    krp = ps1.tile([128, TB], f32, tag="ps", name="krp")
                    for t in range(HT):
                        nc.tensor.matmul(
                            csp[:], wkd_sb[:, t, 0:CDS], hblk[:, t, :],
                            start=(t == 0), stop=(t == HT - 1))
                        nc.tensor.matmul(
                            krp[:], wkd_sb[:, t, CDS:CDS + KRR], hblk[:, t, :],
                            start=(t == 0), stop=(t == HT - 1))
                    cst = st1.tile([128, TB], FP, tag="cst")
                    nc.scalar.copy(cst[:], csp[:])
                    sh, s0 = [(ckv_shA, c0), (ckv_shA, c0),
                              (ckv_shB, 0), (ckv_shC, 0)][blk]
                    nc.sync.dma_start(sh[:, s0:s0 + TB], cst[:])
                    # k-rope rows: per kv head 64 rows = [dims 0:32, dims 64:96]
                    rawk = st1.tile([128, TB], f32, tag="rawk")
                    nc.scalar.copy(rawk[:], krp[:])
                    rotk = st1.tile([128, TB], f32, tag="rotk")
                    nc.sync.dma_start(rotk[0:32, :], rawk[32:64, :])
                    nc.sync.dma_start(rotk[32:64, :], rawk[0:32, :])
                    nc.sync.dma_start(rotk[64:96, :], rawk[96:128, :])
                    nc.sync.dma_start(rotk[96:128, :], rawk[64:96, :])
                    nc.vector.tensor_mul(rawk[:], rawk[:], kcos_sb[:, c0:c1])
                    nc.vector.tensor_mul(rotk[:], rotk[:], ksin_sb[:, c0:c1])
                    for kv in range(KVPC):
                        r0 = kv * 64
                        nc.vector.tensor_add(kT[0:32, kv, c0:c1],
                                             rawk[r0:r0 + 32, :], rotk[r0:r0 + 32, :])
                        nc.vector.tensor_add(kT[64:96, kv, c0:c1],
                                             rawk[r0 + 32:r0 + 64, :], rotk[r0 + 32:r0 + 64, :])

                def emit_q(blk, m):
                    c0, c1 = blk * TB, (blk + 1) * TB
                    hblk = hblks[blk]
                    qp = ps1.tile([128, TB], f32, tag="ps", name=f"qp{m}")
                    for t in range(HT):
                        nc.tensor.matmul(
                            qp[:], wq_sb[:, t, m * 128:(m + 1) * 128],
                            hblk[:, t, :], start=(t == 0), stop=(t == HT - 1))
                    raw = st1.tile([128, TB], f32, tag="qraw")
                    nc.scalar.copy(raw[:], qp[:])
                    rot = st1.tile([128, TB], f32, tag="qrot")
                    nc.sync.dma_start(rot[0:64, :], raw[64:128, :])
                    nc.sync.dma_start(rot[64:128, :], raw[0:64, :])
                    nc.vector.tensor_mul(raw[:], raw[:], qcos_sb[:, c0:c1])
                    nc.vector.tensor_mul(rot[:], rot[:], qsin_sb[:, c0:c1])
                    qsb = st1.tile([128, TB], FP, tag="qsb")
                    nc.vector.tensor_add(qsb[:], raw[:], rot[:])
                    nc.sync.dma_start(qT_s[m, :, c0:c1], qsb[:])

                def emit_up(blk):
                    c0, c1 = blk * TB, (blk + 1) * TB
                    g, g0 = [(ckv_gA, 0), (ckv_gA, TB),
                             (ckv_gB, 0), (ckv_gC, 0)][blk]
                    cb = cbp.tile([128, LT, TB], FP, tag="cb")
                    nc.sync.dma_start(
                        cb[:], g[:, :, g0:g0 + TB].rearrange("g p w -> p g w"))
                    kcp = ps1.tile([128, TB], f32, tag="ps", name="kcp")
                    for lt in range(LT):
                        nc.tensor.matmul(kcp[:], wupk_sb[:, lt, :], cb[:, lt, :],
                                         start=(lt == 0), stop=(lt == LT - 1))
                    # rows: [kv0 d32:64, kv0 d96:128, kv1 d32:64, kv1 d96:128]
                    for kv in range(KVPC):
                        r0 = kv * 64
                        nc.scalar.copy(kT[32:64, kv, c0:c1], kcp[r0:r0 + 32, :])
                        nc.scalar.copy(kT[96:128, kv, c0:c1], kcp[r0 + 32:r0 + 64, :])
                    for tt in range(TB // 128):
                        vp = ps1.tile([128, TB], f32, tag="ps", name=f"vp{tt}")
                        for lt in range(LT):
                            nc.tensor.matmul(
                                vp[:, 0:KVPC * HEAD_DIM],
                                cb[:, lt, tt * 128:(tt + 1) * 128],
                                wupv_sb[:, lt, :],
                                start=(lt == 0), stop=(lt == LT - 1))
                        nc.scalar.copy(v_sb[:, blk * (TB // 128) + tt, :, :],
                                       vp[:, 0:KVPC * HEAD_DIM])

                for blk in range(NTB):
                    if blk + 1 < NTB:
                        c0n = (blk + 1) * TB
                        nc.sync.dma_start(
                            hblks[blk + 1][:],
                            hidT[:, c0n:c0n + TB].rearrange("(t p) w -> p t w", p=128))
                    emit_kd(blk)
                    # gathers issue as soon as their shard blocks are written,
                    # overlapping the q sub-passes
                    if blk == 1:
                        nc.gpsimd.collective_compute(
                            "AllGather", mybir.AluOpType.bypass, cc_groups,
                            ins=[ckv_shA], outs=[ckv_gA])
                    elif blk == 2:
                        nc.gpsimd.collective_compute(
                            "AllGather", mybir.AluOpType.bypass, cc_groups,
                            ins=[ckv_shB], outs=[ckv_gB])
                    elif blk == 3:
                        nc.gpsimd.collective_compute(
                            "AllGather", mybir.AluOpType.bypass, cc_groups,
                            ins=[ckv_shC], outs=[ckv_gC])
                    for m in range(QT):
                        emit_q(blk, m)
                        # up-projections interleave into late q sub-passes:
                        # blocks 0-1 during block 2, blocks 2-3 during block 3
                        if blk >= 2 and m in (4, 6):
                            emit_up(2 * (blk - 2) + (m - 4) // 2)

            # ---------------- phase 2: attention ----------------
            with tc.tile_pool(name="wop", bufs=1) as wop, \
                 tc.tile_pool(name="att", bufs=1) as ap_:
                wo_sb = wop.tile([128, QT, HIDDEN], FP, tag="wo")
                attn_sb = ap_.tile([128, QT, NT], FP, tag="attn")

                with tc.tile_pool(name="cst", bufs=1) as cst2, \
                     tc.tile_pool(name="qh", bufs=8) as qhp, \
                     tc.tile_pool(name="pt", bufs=3) as ptp, \
                     tc.tile_pool(name="sac", bufs=2) as sap, \
                     tc.tile_pool(name="sm", bufs=2) as smp, \
                     tc.tile_pool(name="st4", bufs=4) as st4, \
                     tc.tile_pool(name="psS", bufs=2, space=MS.PSUM) as psS, \
                     tc.tile_pool(name="psO", bufs=3, space=MS.PSUM) as psO, \
                     tc.tile_pool(name="psU", bufs=1, space=MS.PSUM) as psU:
                    masks_sb = cst2.tile([128, NJ, QB], FP, tag="masks")
                    nc.sync.dma_start(masks_sb[:], masks)
                    ones_sb = cst2.tile([128, 1], FP, tag="ones")
                    nc.sync.dma_start(ones_sb[:], onesd)
                    qhs = [qhp.tile([128, NT], FP, tag="qh", name=f"qh{_m}")
                           for _m in range(QT)]
                    for _m in range(QT):
                        nc.sync.dma_start(qhs[_m][:], qT_s[_m])
                    nc.sync.dma_start(
                        wo_sb[:], wo.rearrange("(t p) w -> p t w", p=128))

                    def emit_oproj(n, j):
                        ps = psO.tile([128, QB], f32, tag="ops", name=f"op{n}_{j}")
                        for h2 in range(QT):
                            nc.tensor.matmul(
                                ps[:],
                                wo_sb[:, h2, n * 128:(n + 1) * 128],
                                attn_sb[:, h2, j * QB:(j + 1) * QB],
                                start=(h2 == 0), stop=(h2 == QT - 1))
                        osb = st4.tile([128, QB], FP, tag="osb")
                        nc.vector.tensor_copy(osb[:], ps[:])
                        nc.sync.dma_start(
                            outp[n * 128:(n + 1) * 128, j * QB:(j + 1) * QB],
                            osb[:])

                    # q-block-outer attention; o_proj for token block j
                    # interleaves into block j+1's attention
                    for qb in range(NQB):
                        nkt = (qb + 1) * NJ
                        ubank = psU.tile([128, QB], f32, tag="sps")
                        for m in range(QT):
                            kvh = m // (HPC // KVPC)
                            qh = qhs[m]
                            ops = psO.tile([128, QB], f32, tag="ops",
                                           name=f"ops{m}")
                            sacc = sap.tile([128, QB], FP, tag="sacc")
                            for p2 in range(nkt // 2):
                                scp = psS.tile([128, 2, QB], f32, tag="scp")
                                for h in range(2):
                                    kt = 2 * p2 + h
                                    nc.tensor.matmul(
                                        scp[:, h, :],
                                        kT[:, kvh, kt * 128:(kt + 1) * 128],
                                        qh[:, qb * QB:(qb + 1) * QB],
                                        start=True, stop=True)
                                pt = ptp.tile([128, 2, QB], FP, tag="pt")
                                nc.scalar.activation(pt[:], scp[:], EXP)
                                jj = 2 * p2 - qb * NJ
                                if jj >= 0:
                                    nc.vector.tensor_mul(pt[:], pt[:],
                                                         masks_sb[:, jj:jj + 2, :])
                                for h in range(2):
                                    kt = 2 * p2 + h
                                    nc.tensor.matmul(
                                        ops[:], v_sb[:, kt, kvh, :], pt[:, h, :],
                                        start=(kt == 0), stop=(kt == nkt - 1))
                                if p2 == 0:
                                    nc.vector.tensor_copy(sacc[:], pt[:, 0, :])
                                else:
                                    nc.vector.tensor_add(sacc[:], sacc[:], pt[:, 0, :])
                                nc.vector.tensor_add(sacc[:], sacc[:], pt[:, 1, :])
                            u0 = (32 * m) % 96    # offsets 0/32/64 cycle
                            sps = ubank[u0:u0 + 1, :]
                            nc.tensor.matmul(sps, ones_sb[:], sacc[:],
                                             start=True, stop=True)
                            srow = smp.tile([1, QB], f32, tag="srow")
                            nc.scalar.copy(srow[:], sps)
                            sbc = smp.tile([128, QB], f32, tag="sbc")
                            nc.gpsimd.partition_broadcast(sbc[:], srow[:])
                            rbc = smp.tile([128, QB], f32, tag="rbc")
                            nc.vector.reciprocal_approx_fast(rbc[:], sbc[:])
                            nc.vector.tensor_mul(
                                attn_sb[:, m, qb * QB:(qb + 1) * QB],
                                ops[:], rbc[:])
                            if qb >= 1:
                                for n in range(4 * m, 4 * m + 4):
                                    emit_oproj(n, qb - 1)
                    for n in range(HT):
                        emit_oproj(n, NQB - 1)

    nc.compile()
    return nc


def make_in_maps(hidden_states, Wq, Wkr, Wdk, Wupk, Wupv, Wo, Sv=S, QB=512):
    """Host-side sharding + layout prep. Returns per-core input dicts."""
    NJ = QB // 128
    scale = 1.0 / np.sqrt(np.float32(HEAD_DIM))
    hidden_states = np.asarray(hidden_states, np.float32)
    Wq, Wkr, Wdk = np.asarray(Wq, np.float32), np.asarray(Wkr, np.float32), np.asarray(Wdk, np.float32)
    Wupk, Wupv, Wo = np.asarray(Wupk, np.float32), np.asarray(Wupv, np.float32), np.asarray(Wo, np.float32)

    cos_t, sin_t = _rope_tables(Sv)                    # [128, S]
    qcos = np.ascontiguousarray(cos_t * scale).astype(F16)
    qsin = np.ascontiguousarray(
        np.concatenate([-sin_t[0:64], sin_t[64:128]], axis=0) * scale).astype(F16)
    # per kv head 64 rope rows = [dims 0:32, dims 64:96], tiled x KVPC
    kc1 = np.concatenate([cos_t[0:32], cos_t[64:96]], axis=0)
    ks1 = np.concatenate([-sin_t[0:32], sin_t[64:96]], axis=0)
    kcos = np.ascontiguousarray(np.tile(kc1, (KVPC, 1))).astype(F16)
    ksin = np.ascontiguousarray(np.tile(ks1, (KVPC, 1))).astype(F16)

    k_idx = np.arange(128)[:, None]
    q_idx = np.arange(QB)[None, :]
    masks = np.stack(
        [(q_idx >= j * 128 + k_idx).astype(np.float32) for j in range(NJ)],
        axis=1)                                        # [128, NJ, QB]
    masks = np.ascontiguousarray(masks).astype(F16)

    in_maps = []
    for c in range(NCORES):
        b, tp = c // TPG, c % TPG
        hidT = np.ascontiguousarray(hidden_states[b].T).astype(F16)
        wq_t = np.ascontiguousarray(Wq[QR * tp:QR * (tp + 1)].T).astype(F16)
        wkd_t = np.ascontiguousarray(
            np.concatenate([Wdk[CDS * tp:CDS * (tp + 1)],
                            Wkr[KRR * tp:KRR * (tp + 1)]], axis=0).T).astype(F16)
        wupk_t = np.ascontiguousarray(Wupk[KRR * tp:KRR * (tp + 1)].T).astype(F16)
        wupv_t = np.ascontiguousarray(
            Wupv[KVPC * HEAD_DIM * tp:KVPC * HEAD_DIM * (tp + 1)].T).astype(F16)
        wo_t = np.ascontiguousarray(Wo[:, QR * tp:QR * (tp + 1)].T).astype(F16)
        in_maps.append({
            "hidT": hidT, "wq_t": wq_t, "wkd_t": wkd_t,
            "wupk_t": wupk_t, "wupv_t": wupv_t, "wo_t": wo_t,
            "qcos": qcos, "qsin": qsin, "kcos": kcos, "ksin": ksin,
            "masks": masks, "ones": np.ones((128, 1), F16),
        })
    return in_maps


def combine_outputs(res):
    outs = []
    for b in range(B):
        acc = res.results[b * TPG]["out_part"].astype(np.float32)
        for tp in range(1, TPG):
            acc = acc + res.results[b * TPG + tp]["out_part"].astype(np.float32)
        outs.append(acc.T)                             # [S, HIDDEN]
    return np.stack(outs).astype(np.float32)           # [B, S, HIDDEN]


_NC_CACHE = {}


def _get_program(key=(S, 512, 512)):
    if key not in _NC_CACHE:
        _NC_CACHE[key] = build_program(*key)
    return _NC_CACHE[key]


def kernel(hidden_states, Wq, Wkr, Wdk, Wupk, Wupv, Wo):
    from concourse.bass_utils import run_bass_kernel_spmd

    in_maps = make_in_maps(np.asarray(hidden_states), Wq, Wkr, Wdk, Wupk, Wupv, Wo)
    nc = _get_program()
    res = run_bass_kernel_spmd(nc, in_maps, list(range(NCORES)))
    return combine_outputs(res)
